# revision 9
# baseline (speedup 1.0000x reference)
"""Trainium2 Bass kernel for nn_Diffusion_15436112462451.

Strategy: pure data parallelism over the batch (2048 -> 8 cores x 256),
feature-major activations on-chip, fully unrolled 100-step loop.

Per step (per core):
  - 36 bf16 matmuls on PE: split-precision weights (W = Whi + Wlo in bf16,
    two matmuls accumulating in fp32 PSUM) + rank-1 bias matmuls.
  - 3 sigmoid passes on ScalarE (one per hidden layer).
  - 2 fused custom-DVE passes per layer evaluate the exact-mish rational
    completion  mish(z) ~= z * QUAD(t) * CUBIC(t) + beta,  t = sigmoid(-az-d)^2
    (degree-5 minimax fit of tanh(softplus), max err 6.3e-5; beta is folded
    into the next layer's bias on the host).
  - The denoising x-update runs on small [16,256] DVE ops with per-step
    schedule scalars baked in as immediates.

The time-embedding MLP is batch-independent (the timestep is a scalar per
step), so its contribution is precomputed on the host into a [100,256] bias
table and injected via rank-1 bias matmuls.
"""
import sys
import math
import re
import numpy as np

for _p in ('/opt/trn_rl_repo', '/root/.axon_site/_ro/trn_rl_repo'):
    if _p not in sys.path:
        sys.path.insert(0, _p)

import ml_dtypes
from contextlib import ExitStack
import concourse.bass as bass
from concourse import bacc
from concourse import mybir, tile, bass_utils, dve_ops
from concourse.dve_spec import Spec, Src0, Src1, C0, C1, C2, sq, maxx, minn

BF16 = ml_dtypes.bfloat16
NCORES = 8
BATCH = 2048
BPC = BATCH // NCORES          # 256 batch rows per core
T_STEPS = 100
STATE_DIM, ACTION_DIM, HIDDEN, TIME_DIM = 64, 16, 256, 32
KX = ACTION_DIM + STATE_DIM    # 80 rows of W1 used for [x; state]

# --- activation fit constants (deg-5 sigma-poly factorization) ---
A_S = 0.9990298806699722
D_S = -0.0005000143935776705
BETA = 4.708088756431602e-05
QA, QB, QC = -0.21302398380145082, 0.6455208072356895, -0.6201860532189531
MA, MB, MC = -0.9194163848641597, 1.5334239721923986, -1.6124382654378613


# ---------------------------------------------------------------- custom ops
def _register_op(name, spec):
    for op in dve_ops.OPS:
        if op.name == name:
            return op
    op = dve_ops.DveOp(name, spec, False, uops_sha={"v3": "?", "v4": "?"})
    dve_ops.OPS.append(op)
    dve_ops.CUSTOM_DVE_SPECS[name] = spec
    dve_ops._SUB_OPCODE_FOR_NAME[name] = (
        dve_ops._CUSTOM_DVE_ROW_BASE + len(dve_ops.OPS) - 1)
    for ver in ("v3", "v4"):
        try:
            op.compile(ver)
        except ValueError as e:
            op.uops_sha[ver] = re.search(
                r'uops_sha\["' + ver + r'"\]="([0-9a-f]+)"', str(e)).group(1)
        op.compile(ver)
    return op


_t = sq(Src0)
MISH_A = _register_op("MISH_A_DIFF15436", Spec(
    body=Src1 * ((_t * C0 + C1) * _t + C2),
    reference=lambda in0, in1, s0, s1, imm2:
        (in1 * ((s0 * in0.astype(np.float64) ** 2 + s1) * in0.astype(np.float64) ** 2 + imm2)).astype(np.float32),
))
_t2 = sq(Src0)
MISH_B = _register_op("MISH_B_DIFF15436", Spec(
    body=Src1 * ((((_t2 + C0) * _t2 + C1) * _t2) + C2),
    reference=lambda in0, in1, s0, s1, imm2:
        (in1 * ((((in0.astype(np.float64) ** 2 + s0) * in0.astype(np.float64) ** 2 + s1) * in0.astype(np.float64) ** 2) + imm2)).astype(np.float32),
))
CLIPMULADD = _register_op("CLIPMULADD_DIFF15436", Spec(
    body=minn(maxx(Src0, C0), C1) * C2 + Src1,
    reference=lambda in0, in1, s0, s1, imm2:
        (np.minimum(np.maximum(in0, s0), s1) * imm2 + in1).astype(np.float32),
))


# ---------------------------------------------------------------- schedule
def _vp_schedule():
    t = np.arange(1, T_STEPS + 1, dtype=np.float64)
    b_max, b_min = 10.0, 0.1
    alpha = np.exp(-b_min / T_STEPS - 0.5 * (b_max - b_min) * (2 * t - 1) / T_STEPS ** 2)
    betas = 1.0 - alpha
    ac = np.cumprod(1.0 - betas)
    ac_prev = np.concatenate([[1.0], ac[:-1]])
    return {
        'c1': np.sqrt(1.0 / ac).astype(np.float32),
        'c2': np.sqrt(1.0 / ac - 1.0).astype(np.float32),
        'p1': (betas * np.sqrt(ac_prev) / (1.0 - ac)).astype(np.float32),
        'p2': ((1.0 - ac_prev) * np.sqrt(1.0 - betas) / (1.0 - ac)).astype(np.float32),
        'logvar': np.log(np.clip(betas * (1.0 - ac_prev) / (1.0 - ac), 1e-20, None)).astype(np.float32),
    }


def _mish64(v):
    return v * np.tanh(np.logaddexp(0.0, v))


# ---------------------------------------------------------------- bass build
_CACHE = {}


def _build(nsteps=T_STEPS):
    if ('nc', nsteps) in _CACHE:
        return _CACHE[('nc', nsteps)]
    sched = _vp_schedule()
    c1s, c2s, p1s, p2s = sched['c1'], sched['c2'], sched['p1'], sched['p2']

    nc = bacc.Bacc("TRN2", target_bir_lowering=False, debug=False, num_devices=NCORES)
    f32 = mybir.dt.float32
    bf = mybir.dt.bfloat16

    def din(name, shape, dt=f32):
        return nc.dram_tensor(name, shape, dt, kind="ExternalInput").ap()

    d_state = din("state_t", [STATE_DIM, BPC], bf)
    d_xinit = din("x_init_t", [ACTION_DIM, BPC])
    d_noise = din("noise_t", [T_STEPS, ACTION_DIM, BPC])
    d_w1x_hi = din("w1x_hi", [KX, HIDDEN], bf)
    d_w1x_lo = din("w1x_lo", [KX, HIDDEN], bf)
    d_w2_hi = din("w2_hi", [HIDDEN, HIDDEN], bf)
    d_w2_lo = din("w2_lo", [HIDDEN, HIDDEN], bf)
    d_w3_hi = din("w3_hi", [HIDDEN, HIDDEN], bf)
    d_w3_lo = din("w3_lo", [HIDDEN, HIDDEN], bf)
    d_w4_hi = din("w4_hi", [HIDDEN, ACTION_DIM], bf)
    d_w4_lo = din("w4_lo", [HIDDEN, ACTION_DIM], bf)
    d_cont_hi = din("cont_hi", [1, T_STEPS * HIDDEN], bf)
    d_cont_lo = din("cont_lo", [1, T_STEPS * HIDDEN], bf)
    d_b23_hi = din("b23_hi", [1, 2 * HIDDEN], bf)
    d_b23_lo = din("b23_lo", [1, 2 * HIDDEN], bf)
    d_xb = din("xb_t", [ACTION_DIM, T_STEPS])
    d_out = nc.dram_tensor("out_t", [ACTION_DIM, BPC], f32, kind="ExternalOutput").ap()

    with tile.TileContext(nc) as tc, ExitStack() as ctx:
        wp = ctx.enter_context(tc.tile_pool(name="weights", bufs=1))
        ap_ = ctx.enter_context(tc.tile_pool(name="acts", bufs=2))
        sp = ctx.enter_context(tc.tile_pool(name="small", bufs=2))
        np_ = ctx.enter_context(tc.tile_pool(name="noise", bufs=4))
        pp = ctx.enter_context(tc.tile_pool(name="psum", bufs=2, space="PSUM"))

        def wtile(shape, dt, nm, src):
            t = wp.tile(shape, dt, tag=nm, name=nm)
            nc.gpsimd.dma_start(t, src)
            return t

        w1x_hi = wtile([KX, HIDDEN], bf, "w1x_hi", d_w1x_hi)
        w1x_lo = wtile([KX, HIDDEN], bf, "w1x_lo", d_w1x_lo)
        w2 = {}
        w3 = {}
        w4 = {}
        for nm, dhi, dlo, dst in (("w2", d_w2_hi, d_w2_lo, w2),
                                  ("w3", d_w3_hi, d_w3_lo, w3)):
            for hl, dd in (("hi", dhi), ("lo", dlo)):
                for kc in (0, 1):
                    dst[(hl, kc)] = wtile([128, HIDDEN], bf, f"{nm}_{hl}_{kc}",
                                          dd[kc * 128:(kc + 1) * 128, :])
        for hl, dd in (("hi", d_w4_hi), ("lo", d_w4_lo)):
            for kc in (0, 1):
                w4[(hl, kc)] = wtile([128, ACTION_DIM], bf, f"w4_{hl}_{kc}",
                                     dd[kc * 128:(kc + 1) * 128, :])
        cont_hi = wtile([1, T_STEPS * HIDDEN], bf, "cont_hi", d_cont_hi)
        cont_lo = wtile([1, T_STEPS * HIDDEN], bf, "cont_lo", d_cont_lo)
        b23_hi = wtile([1, 2 * HIDDEN], bf, "b23_hi", d_b23_hi)
        b23_lo = wtile([1, 2 * HIDDEN], bf, "b23_lo", d_b23_lo)
        xb = wtile([ACTION_DIM, T_STEPS], f32, "xb", d_xb)

        ones = wp.tile([1, BPC], bf, tag="ones", name="ones")
        nc.vector.memset(ones, 1.0)
        sig_bias = wp.tile([128, 1], f32, tag="sig_bias", name="sig_bias")
        nc.vector.memset(sig_bias, -D_S)

        hT = wp.tile([KX, BPC], bf, tag="hT", name="hT")
        nc.gpsimd.dma_start(hT[ACTION_DIM:KX, :], d_state)
        xT = wp.tile([ACTION_DIM, BPC], f32, tag="xT", name="xT")
        nc.gpsimd.dma_start(xT, d_xinit)

        SIG = mybir.ActivationFunctionType.Sigmoid
        MUL = mybir.AluOpType.mult
        ADD = mybir.AluOpType.add
        MAX = mybir.AluOpType.max
        MIN = mybir.AluOpType.min

        for k in range(nsteps):
            i = T_STEPS - 1 - k
            c1 = float(c1s[i]); c2 = float(c2s[i])
            p1 = float(p1s[i]); p2 = float(p2s[i])

            # bf16 view of x for the L1 matmul
            nc.vector.tensor_copy(hT[0:ACTION_DIM, :], xT)

            # noise for this step (pre-scaled by sigma on the host)
            nz = np_.tile([ACTION_DIM, BPC], f32, tag="nz", name="nz")
            nc.sync.dma_start(nz, d_noise[k])

            # early elementwise pieces (only depend on x_k and noise)
            xs = sp.tile([ACTION_DIM, BPC], f32, tag="xs", name="xs")
            nc.vector.tensor_scalar(xs, xT, c1, xb[:, i:i + 1], MUL, ADD)
            s2 = sp.tile([ACTION_DIM, BPC], f32, tag="s2", name="s2")
            nc.vector.scalar_tensor_tensor(s2, xT, p2, nz, MUL, ADD)

            # ---- the 3 hidden layers ----
            hprev = None
            for L, (wd, bias_off) in enumerate((
                    (None, None), (w2, 0), (w3, HIDDEN))):
                z = pp.tile([128, 2 * BPC], mybir.dt.float32, tag=f"z{L}", name=f"z{L}")
                for mc in (0, 1):
                    zslice = z[:, mc * BPC:(mc + 1) * BPC]
                    if L == 0:
                        off = i * HIDDEN + mc * 128
                        nc.tensor.matmul(zslice, cont_hi[0:1, off:off + 128], ones, start=True, stop=False)
                        nc.tensor.matmul(zslice, cont_lo[0:1, off:off + 128], ones, start=False, stop=False)
                        nc.tensor.matmul(zslice, w1x_hi[:, mc * 128:(mc + 1) * 128], hT, start=False, stop=False)
                        nc.tensor.matmul(zslice, w1x_lo[:, mc * 128:(mc + 1) * 128], hT, start=False, stop=True)
                    else:
                        off = bias_off + mc * 128
                        nc.tensor.matmul(zslice, b23_hi[0:1, off:off + 128], ones, start=True, stop=False)
                        nc.tensor.matmul(zslice, b23_lo[0:1, off:off + 128], ones, start=False, stop=False)
                        for kc in (0, 1):
                            rhs = hprev[:, kc * BPC:(kc + 1) * BPC]
                            nc.tensor.matmul(zslice, wd[("hi", kc)][:, mc * 128:(mc + 1) * 128], rhs, start=False, stop=False)
                            nc.tensor.matmul(zslice, wd[("lo", kc)][:, mc * 128:(mc + 1) * 128], rhs,
                                             start=False, stop=(kc == 1))
                # sigmoid pass: s = sigmoid(-(A_S*z + D_S))
                s = ap_.tile([128, 2 * BPC], mybir.dt.float32, tag="s", name="s")
                nc.scalar.activation(s, z, SIG, bias=sig_bias, scale=-A_S)
                # custom completion: h = z*QUAD(t)*CUBIC(t), t = s^2
                wA = ap_.tile([128, 2 * BPC], mybir.dt.float32, tag="wA", name="wA")
                nc.vector._custom_dve(MISH_A, out=wA, in0=s, in1=z, s0=QA, s1=QB, imm2=QC)
                h = ap_.tile([128, 2 * BPC], bf, tag=f"h{L}", name=f"h{L}")
                nc.vector._custom_dve(MISH_B, out=h, in0=s, in1=wA, s0=MA, s1=MB, imm2=MC)
                hprev = h

            # ---- L4: eps psum [16, BPC] ----
            z4 = pp.tile([ACTION_DIM, BPC], mybir.dt.float32, tag="z4", name="z4")
            nc.tensor.matmul(z4, w4[("hi", 0)], hprev[:, 0:BPC], start=True, stop=False)
            nc.tensor.matmul(z4, w4[("lo", 0)], hprev[:, 0:BPC], start=False, stop=False)
            nc.tensor.matmul(z4, w4[("hi", 1)], hprev[:, BPC:2 * BPC], start=False, stop=False)
            nc.tensor.matmul(z4, w4[("lo", 1)], hprev[:, BPC:2 * BPC], start=False, stop=True)

            # ---- x update ----
            pre = sp.tile([ACTION_DIM, BPC], f32, tag="pre", name="pre")
            nc.vector.scalar_tensor_tensor(pre, z4, -c2, xs, MUL, ADD)
            # xT <- clip(pre, -1, 1)*p1 + s2
            nc.vector._custom_dve(CLIPMULADD, out=xT, in0=pre, in1=s2,
                                  s0=-1.0, s1=1.0, imm2=p1)

        out_f = sp.tile([ACTION_DIM, BPC], f32, tag="out_f", name="out_f")
        nc.vector.tensor_scalar(out_f, xT, -1.0, 1.0, MAX, MIN)
        nc.sync.dma_start(d_out, out_f)

    nc.compile()
    _CACHE[('nc', nsteps)] = nc
    return nc


# ---------------------------------------------------------------- host side
def _host_prep(inputs):
    sched = _vp_schedule()
    f64 = np.float64

    W1 = np.asarray(inputs['W1'], np.float32)
    b1 = np.asarray(inputs['b1'], np.float32)
    W2 = np.asarray(inputs['W2'], np.float32)
    b2 = np.asarray(inputs['b2'], np.float32)
    W3 = np.asarray(inputs['W3'], np.float32)
    b3 = np.asarray(inputs['b3'], np.float32)
    W4 = np.asarray(inputs['W4'], np.float32)
    b4 = np.asarray(inputs['b4'], np.float32)

    # time-embedding MLP for all 100 timesteps (host, float64)
    half = TIME_DIM // 2
    freqs = np.exp(np.arange(half, dtype=f64) * (-math.log(10000.0) / (half - 1)))
    ivals = np.arange(T_STEPS, dtype=f64)
    ang = ivals[:, None] * freqs[None, :]
    emb = np.concatenate([np.sin(ang), np.cos(ang)], axis=1)
    t1 = _mish64(emb @ np.asarray(inputs['time_W1'], f64) + np.asarray(inputs['time_b1'], f64))
    temb = t1 @ np.asarray(inputs['time_W2'], f64) + np.asarray(inputs['time_b2'], f64)

    # beta-folded biases
    b2e = b2.astype(f64) + BETA * W2.astype(f64).sum(axis=0)
    b3e = b3.astype(f64) + BETA * W3.astype(f64).sum(axis=0)
    b4e = b4.astype(f64) + BETA * W4.astype(f64).sum(axis=0)

    # contrib[i] = temb[i] @ W1[16:48] + b1   -> flat [1, 100*256]
    contrib = (temb @ W1[16:48].astype(f64) + b1.astype(f64))  # [100, 256]

    def hilo(v):
        v32 = np.asarray(v, np.float32)
        hi = v32.astype(BF16)
        lo = (v32 - hi.astype(np.float32)).astype(BF16)
        return hi, lo

    cont_hi, cont_lo = hilo(contrib.reshape(1, -1))
    b23_hi, b23_lo = hilo(np.concatenate([b2e, b3e]).reshape(1, -1))
    w1x = np.concatenate([W1[0:16], W1[48:112]], axis=0)
    w1x_hi, w1x_lo = hilo(w1x)
    w2_hi, w2_lo = hilo(W2)
    w3_hi, w3_lo = hilo(W3)
    w4_hi, w4_lo = hilo(W4)

    # x-update tables
    xb = (-sched['c2'].astype(f64)[None, :] * b4e[:, None]).astype(np.float32)  # [16, 100]

    # per-step noise scaling (fp32, matching the reference ops)
    sig = np.exp(0.5 * sched['logvar']).astype(np.float32)  # [100] by timestep i
    ik = (T_STEPS - 1 - np.arange(T_STEPS))                 # timestep for step k
    scale = sig[ik] * (ik != 0).astype(np.float32)          # [100]
    noise = np.asarray(inputs['noise'], np.float32)
    noise_scaled = noise * scale[:, None, None]

    state = np.asarray(inputs['state'], np.float32)
    x_init = np.asarray(inputs['x_init'], np.float32)

    shared = dict(
        w1x_hi=w1x_hi, w1x_lo=w1x_lo, w2_hi=w2_hi, w2_lo=w2_lo,
        w3_hi=w3_hi, w3_lo=w3_lo, w4_hi=w4_hi, w4_lo=w4_lo,
        cont_hi=cont_hi, cont_lo=cont_lo, b23_hi=b23_hi, b23_lo=b23_lo,
        xb_t=xb,
    )
    in_maps = []
    for c in range(NCORES):
        sl = slice(c * BPC, (c + 1) * BPC)
        m = dict(shared)
        m['state_t'] = np.ascontiguousarray(state[sl].T).astype(BF16)
        m['x_init_t'] = np.ascontiguousarray(x_init[sl].T)
        m['noise_t'] = np.ascontiguousarray(noise_scaled[:, sl, :].transpose(0, 2, 1))
        in_maps.append(m)
    return in_maps


def run(inputs, trace=False, nsteps=T_STEPS):
    nc = _build(nsteps)
    in_maps = _host_prep(inputs)
    res = bass_utils.run_bass_kernel_spmd(
        nc, in_maps, core_ids=list(range(NCORES)), trace=trace)
    out = np.empty((BATCH, ACTION_DIM), np.float32)
    for c in range(NCORES):
        out[c * BPC:(c + 1) * BPC] = res.results[c]['out_t'].T
    return out, res


def kernel(**inputs) -> np.ndarray:
    out, _ = run(inputs, trace=False)
    return out


# revision 10
# speedup vs baseline: 1.3495x; 1.3495x over previous
"""Trainium2 Bass kernel for nn_Diffusion_15436112462451.

Strategy: pure data parallelism over the batch (2048 -> 8 cores x 256),
feature-major activations on-chip, fully unrolled 100-step loop.

Per step (per core):
  - 36 bf16 matmuls on PE: split-precision weights (W = Whi + Wlo in bf16,
    two matmuls accumulating in fp32 PSUM) + rank-1 bias matmuls.
  - 3 sigmoid passes on ScalarE (one per hidden layer).
  - 2 fused custom-DVE passes per layer evaluate the exact-mish rational
    completion  mish(z) ~= z * QUAD(t) * CUBIC(t) + beta,  t = sigmoid(-az-d)^2
    (degree-5 minimax fit of tanh(softplus), max err 6.3e-5; beta is folded
    into the next layer's bias on the host).
  - The denoising x-update runs on small [16,256] DVE ops with per-step
    schedule scalars baked in as immediates.

The time-embedding MLP is batch-independent (the timestep is a scalar per
step), so its contribution is precomputed on the host into a [100,256] bias
table and injected via rank-1 bias matmuls.
"""
import sys
import math
import re
import numpy as np

for _p in ('/opt/trn_rl_repo', '/root/.axon_site/_ro/trn_rl_repo'):
    if _p not in sys.path:
        sys.path.insert(0, _p)

import ml_dtypes
from contextlib import ExitStack
import concourse.bass as bass
from concourse import bacc
from concourse import mybir, tile, bass_utils, dve_ops
from concourse.dve_spec import Spec, Src0, Src1, C0, C1, C2, sq, maxx, minn

BF16 = ml_dtypes.bfloat16
NCORES = 8
BATCH = 2048
BPC = BATCH // NCORES          # 256 batch rows per core
T_STEPS = 100
STATE_DIM, ACTION_DIM, HIDDEN, TIME_DIM = 64, 16, 256, 32
KX = ACTION_DIM + STATE_DIM    # 80 rows of W1 used for [x; state]

# --- activation fit constants (deg-5 sigma-poly factorization) ---
A_S = 0.9990298806699722
D_S = -0.0005000143935776705
BETA = 4.708088756431602e-05
QA, QB, QC = -0.21302398380145082, 0.6455208072356895, -0.6201860532189531
MA, MB, MC = -0.9194163848641597, 1.5334239721923986, -1.6124382654378613


# ---------------------------------------------------------------- custom ops
def _register_op(name, spec):
    for op in dve_ops.OPS:
        if op.name == name:
            return op
    op = dve_ops.DveOp(name, spec, False, uops_sha={"v3": "?", "v4": "?"})
    dve_ops.OPS.append(op)
    dve_ops.CUSTOM_DVE_SPECS[name] = spec
    dve_ops._SUB_OPCODE_FOR_NAME[name] = (
        dve_ops._CUSTOM_DVE_ROW_BASE + len(dve_ops.OPS) - 1)
    for ver in ("v3", "v4"):
        try:
            op.compile(ver)
        except ValueError as e:
            op.uops_sha[ver] = re.search(
                r'uops_sha\["' + ver + r'"\]="([0-9a-f]+)"', str(e)).group(1)
        op.compile(ver)
    return op


_t = sq(Src0)
MISH_A = _register_op("MISH_A_DIFF15436", Spec(
    body=Src1 * ((_t * C0 + C1) * _t + C2),
    reference=lambda in0, in1, s0, s1, imm2:
        (in1 * ((s0 * in0.astype(np.float64) ** 2 + s1) * in0.astype(np.float64) ** 2 + imm2)).astype(np.float32),
))
_t2 = sq(Src0)
MISH_B = _register_op("MISH_B_DIFF15436", Spec(
    body=Src1 * ((((_t2 + C0) * _t2 + C1) * _t2) + C2),
    reference=lambda in0, in1, s0, s1, imm2:
        (in1 * ((((in0.astype(np.float64) ** 2 + s0) * in0.astype(np.float64) ** 2 + s1) * in0.astype(np.float64) ** 2) + imm2)).astype(np.float32),
))
PREOP = _register_op("PREOP_DIFF15436", Spec(
    body=Src0 * C2 + Src1 * C1 + C0,
    reference=lambda in0, in1, s0, s1, imm2:
        (in0 * imm2 + in1 * s1 + s0).astype(np.float32),
))
CLIPMULADD = _register_op("CLIPMULADD_DIFF15436", Spec(
    body=minn(maxx(Src0, C0), C1) * C2 + Src1,
    reference=lambda in0, in1, s0, s1, imm2:
        (np.minimum(np.maximum(in0, s0), s1) * imm2 + in1).astype(np.float32),
))


# ---------------------------------------------------------------- schedule
def _vp_schedule():
    t = np.arange(1, T_STEPS + 1, dtype=np.float64)
    b_max, b_min = 10.0, 0.1
    alpha = np.exp(-b_min / T_STEPS - 0.5 * (b_max - b_min) * (2 * t - 1) / T_STEPS ** 2)
    betas = 1.0 - alpha
    ac = np.cumprod(1.0 - betas)
    ac_prev = np.concatenate([[1.0], ac[:-1]])
    return {
        'c1': np.sqrt(1.0 / ac).astype(np.float32),
        'c2': np.sqrt(1.0 / ac - 1.0).astype(np.float32),
        'p1': (betas * np.sqrt(ac_prev) / (1.0 - ac)).astype(np.float32),
        'p2': ((1.0 - ac_prev) * np.sqrt(1.0 - betas) / (1.0 - ac)).astype(np.float32),
        'logvar': np.log(np.clip(betas * (1.0 - ac_prev) / (1.0 - ac), 1e-20, None)).astype(np.float32),
    }


def _mish64(v):
    return v * np.tanh(np.logaddexp(0.0, v))


# ---------------------------------------------------------------- bass build
_CACHE = {}


def _build(nsteps=T_STEPS):
    if ('nc', nsteps) in _CACHE:
        return _CACHE[('nc', nsteps)]
    sched = _vp_schedule()
    c1s, c2s, p1s, p2s = sched['c1'], sched['c2'], sched['p1'], sched['p2']

    nc = bacc.Bacc("TRN2", target_bir_lowering=False, debug=False, num_devices=NCORES)
    f32 = mybir.dt.float32
    bf = mybir.dt.bfloat16

    def din(name, shape, dt=f32):
        return nc.dram_tensor(name, shape, dt, kind="ExternalInput").ap()

    d_state = din("state_t", [STATE_DIM, BPC], bf)
    d_xinit = din("x_init_t", [ACTION_DIM, BPC])
    d_noise = din("noise_t", [T_STEPS, ACTION_DIM, BPC])
    d_w1x_hi = din("w1x_hi", [KX, HIDDEN], bf)
    d_w1x_lo = din("w1x_lo", [KX, HIDDEN], bf)
    d_w2_hi = din("w2_hi", [HIDDEN, HIDDEN], bf)
    d_w2_lo = din("w2_lo", [HIDDEN, HIDDEN], bf)
    d_w3_hi = din("w3_hi", [HIDDEN, HIDDEN], bf)
    d_w3_lo = din("w3_lo", [HIDDEN, HIDDEN], bf)
    d_w4_hi = din("w4_hi", [HIDDEN, ACTION_DIM], bf)
    d_w4_lo = din("w4_lo", [HIDDEN, ACTION_DIM], bf)
    d_cont = din("cont_hl", [2, T_STEPS * HIDDEN], bf)
    d_b23 = din("b23_hl", [2, 2 * HIDDEN], bf)
    d_xb = din("xb_t", [ACTION_DIM, T_STEPS])
    d_out = nc.dram_tensor("out_t", [ACTION_DIM, BPC], f32, kind="ExternalOutput").ap()

    with tile.TileContext(nc) as tc, ExitStack() as ctx:
        wp = ctx.enter_context(tc.tile_pool(name="weights", bufs=1))
        ap_ = ctx.enter_context(tc.tile_pool(name="acts", bufs=2))
        sp = ctx.enter_context(tc.tile_pool(name="small", bufs=2))
        np_ = ctx.enter_context(tc.tile_pool(name="noise", bufs=4))
        pp = ctx.enter_context(tc.tile_pool(name="psum", bufs=2, space="PSUM"))

        def wtile(shape, dt, nm, src):
            t = wp.tile(shape, dt, tag=nm, name=nm)
            nc.gpsimd.dma_start(t, src)
            return t

        w1x_hi = wtile([KX, HIDDEN], bf, "w1x_hi", d_w1x_hi)
        w1x_lo = wtile([KX, HIDDEN], bf, "w1x_lo", d_w1x_lo)
        w2 = {}
        w3 = {}
        w4 = {}
        for nm, dhi, dlo, dst in (("w2", d_w2_hi, d_w2_lo, w2),
                                  ("w3", d_w3_hi, d_w3_lo, w3)):
            for hl, dd in (("hi", dhi), ("lo", dlo)):
                for kc in (0, 1):
                    dst[(hl, kc)] = wtile([128, HIDDEN], bf, f"{nm}_{hl}_{kc}",
                                          dd[kc * 128:(kc + 1) * 128, :])
        for hl, dd in (("hi", d_w4_hi), ("lo", d_w4_lo)):
            for kc in (0, 1):
                w4[(hl, kc)] = wtile([128, ACTION_DIM], bf, f"w4_{hl}_{kc}",
                                     dd[kc * 128:(kc + 1) * 128, :])
        cont = wtile([2, T_STEPS * HIDDEN], bf, "cont", d_cont)
        b23 = wtile([2, 2 * HIDDEN], bf, "b23", d_b23)
        xb = wtile([ACTION_DIM, T_STEPS], f32, "xb", d_xb)

        ones = wp.tile([2, BPC], bf, tag="ones", name="ones")
        nc.vector.memset(ones, 1.0)
        sig_bias = wp.tile([128, 1], f32, tag="sig_bias", name="sig_bias")
        nc.vector.memset(sig_bias, -D_S)

        hT = wp.tile([KX, BPC], bf, tag="hT", name="hT")
        nc.gpsimd.dma_start(hT[ACTION_DIM:KX, :], d_state)
        xT = wp.tile([ACTION_DIM, BPC], f32, tag="xT", name="xT")
        nc.gpsimd.dma_start(xT, d_xinit)

        SIG = mybir.ActivationFunctionType.Sigmoid
        MUL = mybir.AluOpType.mult
        ADD = mybir.AluOpType.add
        MAX = mybir.AluOpType.max
        MIN = mybir.AluOpType.min

        for k in range(nsteps):
            i = T_STEPS - 1 - k
            c1 = float(c1s[i]); c2 = float(c2s[i])
            p1 = float(p1s[i]); p2 = float(p2s[i])

            # bf16 view of x for the L1 matmul
            nc.vector.tensor_copy(hT[0:ACTION_DIM, :], xT)

            # noise for this step (pre-scaled by sigma on the host)
            nz = np_.tile([ACTION_DIM, BPC], f32, tag="nz", name="nz")
            nc.sync.dma_start(nz, d_noise[k])

            # early elementwise pieces (only depend on x_k and noise)
            s2 = sp.tile([ACTION_DIM, BPC], f32, tag="s2", name="s2")
            nc.vector.scalar_tensor_tensor(s2, xT, p2, nz, MUL, ADD)

            # ---- the 3 hidden layers ----
            hprev = None
            for L, (wd, bias_off) in enumerate((
                    (None, None), (w2, 0), (w3, HIDDEN))):
                z = pp.tile([128, 2 * BPC], mybir.dt.float32, tag=f"z{L}", name=f"z{L}")
                for mc in (0, 1):
                    zslice = z[:, mc * BPC:(mc + 1) * BPC]
                    if L == 0:
                        off = i * HIDDEN + mc * 128
                        nc.tensor.matmul(zslice, cont[0:2, off:off + 128], ones, start=True, stop=False)
                        nc.tensor.matmul(zslice, w1x_hi[:, mc * 128:(mc + 1) * 128], hT, start=False, stop=False)
                        nc.tensor.matmul(zslice, w1x_lo[:, mc * 128:(mc + 1) * 128], hT, start=False, stop=True)
                    else:
                        off = bias_off + mc * 128
                        nc.tensor.matmul(zslice, b23[0:2, off:off + 128], ones, start=True, stop=False)
                        for kc in (0, 1):
                            rhs = hprev[:, kc * BPC:(kc + 1) * BPC]
                            nc.tensor.matmul(zslice, wd[("hi", kc)][:, mc * 128:(mc + 1) * 128], rhs, start=False, stop=False)
                            nc.tensor.matmul(zslice, wd[("lo", kc)][:, mc * 128:(mc + 1) * 128], rhs,
                                             start=False, stop=(kc == 1))
                # sigmoid pass: s = sigmoid(-(A_S*z + D_S))
                s = ap_.tile([128, 2 * BPC], mybir.dt.float32, tag="s", name="s")
                nc.scalar.activation(s, z, SIG, bias=sig_bias, scale=-A_S)
                # custom completion: h = z*QUAD(t)*CUBIC(t), t = s^2
                wA = ap_.tile([128, 2 * BPC], mybir.dt.float32, tag="wA", name="wA")
                nc.vector._custom_dve(MISH_A, out=wA, in0=s, in1=z, s0=QA, s1=QB, imm2=QC)
                h = ap_.tile([128, 2 * BPC], bf, tag=f"h{L}", name=f"h{L}")
                nc.vector._custom_dve(MISH_B, out=h, in0=s, in1=wA, s0=MA, s1=MB, imm2=MC)
                hprev = h

            # ---- L4: eps psum [16, BPC] ----
            z4 = pp.tile([ACTION_DIM, BPC], mybir.dt.float32, tag="z4", name="z4")
            nc.tensor.matmul(z4, w4[("hi", 0)], hprev[:, 0:BPC], start=True, stop=False)
            nc.tensor.matmul(z4, w4[("lo", 0)], hprev[:, 0:BPC], start=False, stop=False)
            nc.tensor.matmul(z4, w4[("hi", 1)], hprev[:, BPC:2 * BPC], start=False, stop=False)
            nc.tensor.matmul(z4, w4[("lo", 1)], hprev[:, BPC:2 * BPC], start=False, stop=True)

            # ---- x update ----
            pre = sp.tile([ACTION_DIM, BPC], f32, tag="pre", name="pre")
            nc.vector._custom_dve(PREOP, out=pre, in0=z4, in1=xT,
                                  s0=xb[:, i:i + 1], s1=c1, imm2=-c2)
            # xT <- clip(pre, -1, 1)*p1 + s2
            nc.vector._custom_dve(CLIPMULADD, out=xT, in0=pre, in1=s2,
                                  s0=-1.0, s1=1.0, imm2=p1)

        out_f = sp.tile([ACTION_DIM, BPC], f32, tag="out_f", name="out_f")
        nc.vector.tensor_scalar(out_f, xT, -1.0, 1.0, MAX, MIN)
        nc.sync.dma_start(d_out, out_f)

    nc.compile()
    _CACHE[('nc', nsteps)] = nc
    return nc


# ---------------------------------------------------------------- host side
def _host_prep(inputs):
    sched = _vp_schedule()
    f64 = np.float64

    W1 = np.asarray(inputs['W1'], np.float32)
    b1 = np.asarray(inputs['b1'], np.float32)
    W2 = np.asarray(inputs['W2'], np.float32)
    b2 = np.asarray(inputs['b2'], np.float32)
    W3 = np.asarray(inputs['W3'], np.float32)
    b3 = np.asarray(inputs['b3'], np.float32)
    W4 = np.asarray(inputs['W4'], np.float32)
    b4 = np.asarray(inputs['b4'], np.float32)

    # time-embedding MLP for all 100 timesteps (host, float64)
    half = TIME_DIM // 2
    freqs = np.exp(np.arange(half, dtype=f64) * (-math.log(10000.0) / (half - 1)))
    ivals = np.arange(T_STEPS, dtype=f64)
    ang = ivals[:, None] * freqs[None, :]
    emb = np.concatenate([np.sin(ang), np.cos(ang)], axis=1)
    t1 = _mish64(emb @ np.asarray(inputs['time_W1'], f64) + np.asarray(inputs['time_b1'], f64))
    temb = t1 @ np.asarray(inputs['time_W2'], f64) + np.asarray(inputs['time_b2'], f64)

    # beta-folded biases
    b2e = b2.astype(f64) + BETA * W2.astype(f64).sum(axis=0)
    b3e = b3.astype(f64) + BETA * W3.astype(f64).sum(axis=0)
    b4e = b4.astype(f64) + BETA * W4.astype(f64).sum(axis=0)

    # contrib[i] = temb[i] @ W1[16:48] + b1   -> flat [1, 100*256]
    contrib = (temb @ W1[16:48].astype(f64) + b1.astype(f64))  # [100, 256]

    def hilo(v):
        v32 = np.asarray(v, np.float32)
        hi = v32.astype(BF16)
        lo = (v32 - hi.astype(np.float32)).astype(BF16)
        return hi, lo

    c_hi, c_lo = hilo(contrib.reshape(1, -1))
    cont_hl = np.concatenate([c_hi, c_lo], axis=0)
    bb_hi, bb_lo = hilo(np.concatenate([b2e, b3e]).reshape(1, -1))
    b23_hl = np.concatenate([bb_hi, bb_lo], axis=0)
    w1x = np.concatenate([W1[0:16], W1[48:112]], axis=0)
    w1x_hi, w1x_lo = hilo(w1x)
    w2_hi, w2_lo = hilo(W2)
    w3_hi, w3_lo = hilo(W3)
    w4_hi, w4_lo = hilo(W4)

    # x-update tables
    xb = (-sched['c2'].astype(f64)[None, :] * b4e[:, None]).astype(np.float32)  # [16, 100]

    # per-step noise scaling (fp32, matching the reference ops)
    sig = np.exp(0.5 * sched['logvar']).astype(np.float32)  # [100] by timestep i
    ik = (T_STEPS - 1 - np.arange(T_STEPS))                 # timestep for step k
    scale = sig[ik] * (ik != 0).astype(np.float32)          # [100]
    noise = np.asarray(inputs['noise'], np.float32)
    noise_scaled = noise * scale[:, None, None]

    state = np.asarray(inputs['state'], np.float32)
    x_init = np.asarray(inputs['x_init'], np.float32)

    shared = dict(
        w1x_hi=w1x_hi, w1x_lo=w1x_lo, w2_hi=w2_hi, w2_lo=w2_lo,
        w3_hi=w3_hi, w3_lo=w3_lo, w4_hi=w4_hi, w4_lo=w4_lo,
        cont_hl=cont_hl, b23_hl=b23_hl,
        xb_t=xb,
    )
    in_maps = []
    for c in range(NCORES):
        sl = slice(c * BPC, (c + 1) * BPC)
        m = dict(shared)
        m['state_t'] = np.ascontiguousarray(state[sl].T).astype(BF16)
        m['x_init_t'] = np.ascontiguousarray(x_init[sl].T)
        m['noise_t'] = np.ascontiguousarray(noise_scaled[:, sl, :].transpose(0, 2, 1))
        in_maps.append(m)
    return in_maps


def run(inputs, trace=False, nsteps=T_STEPS):
    nc = _build(nsteps)
    in_maps = _host_prep(inputs)
    res = bass_utils.run_bass_kernel_spmd(
        nc, in_maps, core_ids=list(range(NCORES)), trace=trace)
    out = np.empty((BATCH, ACTION_DIM), np.float32)
    for c in range(NCORES):
        out[c * BPC:(c + 1) * BPC] = res.results[c]['out_t'].T
    return out, res


def kernel(**inputs) -> np.ndarray:
    out, _ = run(inputs, trace=False)
    return out


# revision 11
# speedup vs baseline: 1.4036x; 1.0401x over previous
"""Trainium2 Bass kernel for nn_Diffusion_15436112462451.

Strategy: pure data parallelism over the batch (2048 -> 8 cores x 256),
feature-major activations on-chip, fully unrolled 100-step loop.

Per step (per core):
  - 36 bf16 matmuls on PE: split-precision weights (W = Whi + Wlo in bf16,
    two matmuls accumulating in fp32 PSUM) + rank-1 bias matmuls.
  - 3 sigmoid passes on ScalarE (one per hidden layer).
  - 2 fused custom-DVE passes per layer evaluate the exact-mish rational
    completion  mish(z) ~= z * QUAD(t) * CUBIC(t) + beta,  t = sigmoid(-az-d)^2
    (degree-5 minimax fit of tanh(softplus), max err 6.3e-5; beta is folded
    into the next layer's bias on the host).
  - The denoising x-update runs on small [16,256] DVE ops with per-step
    schedule scalars baked in as immediates.

The time-embedding MLP is batch-independent (the timestep is a scalar per
step), so its contribution is precomputed on the host into a [100,256] bias
table and injected via rank-1 bias matmuls.
"""
import sys
import math
import re
import numpy as np

for _p in ('/opt/trn_rl_repo', '/root/.axon_site/_ro/trn_rl_repo'):
    if _p not in sys.path:
        sys.path.insert(0, _p)

import ml_dtypes
from contextlib import ExitStack
import concourse.bass as bass
from concourse import bacc
from concourse import mybir, tile, bass_utils, dve_ops
from concourse.dve_spec import Spec, Src0, Src1, C0, C1, C2, sq, maxx, minn

BF16 = ml_dtypes.bfloat16
NCORES = 8
BATCH = 2048
BPC = BATCH // NCORES          # 256 batch rows per core
T_STEPS = 100
STATE_DIM, ACTION_DIM, HIDDEN, TIME_DIM = 64, 16, 256, 32
KX = ACTION_DIM + STATE_DIM    # 80 rows of W1 used for [x; state]

# --- activation fit constants (deg-5 sigma-poly factorization) ---
A_S = 0.9990298806699722
D_S = -0.0005000143935776705
BETA = 4.708088756431602e-05
QA, QB, QC = -0.21302398380145082, 0.6455208072356895, -0.6201860532189531
MA, MB, MC = -0.9194163848641597, 1.5334239721923986, -1.6124382654378613


# ---------------------------------------------------------------- custom ops
def _register_op(name, spec):
    for op in dve_ops.OPS:
        if op.name == name:
            return op
    op = dve_ops.DveOp(name, spec, False, uops_sha={"v3": "?", "v4": "?"})
    dve_ops.OPS.append(op)
    dve_ops.CUSTOM_DVE_SPECS[name] = spec
    dve_ops._SUB_OPCODE_FOR_NAME[name] = (
        dve_ops._CUSTOM_DVE_ROW_BASE + len(dve_ops.OPS) - 1)
    for ver in ("v3", "v4"):
        try:
            op.compile(ver)
        except ValueError as e:
            op.uops_sha[ver] = re.search(
                r'uops_sha\["' + ver + r'"\]="([0-9a-f]+)"', str(e)).group(1)
        op.compile(ver)
    return op


_t = sq(Src0)
MISH_A = _register_op("MISH_A_DIFF15436", Spec(
    body=Src1 * ((_t * C0 + C1) * _t + C2),
    reference=lambda in0, in1, s0, s1, imm2:
        (in1 * ((s0 * in0.astype(np.float64) ** 2 + s1) * in0.astype(np.float64) ** 2 + imm2)).astype(np.float32),
))
_t2 = sq(Src0)
MISH_B = _register_op("MISH_B_DIFF15436", Spec(
    body=Src1 * ((((_t2 + C0) * _t2 + C1) * _t2) + C2),
    reference=lambda in0, in1, s0, s1, imm2:
        (in1 * ((((in0.astype(np.float64) ** 2 + s0) * in0.astype(np.float64) ** 2 + s1) * in0.astype(np.float64) ** 2) + imm2)).astype(np.float32),
))
PREOP = _register_op("PREOP_DIFF15436", Spec(
    body=Src0 * C2 + Src1 * C1 + C0,
    reference=lambda in0, in1, s0, s1, imm2:
        (in0 * imm2 + in1 * s1 + s0).astype(np.float32),
))
CLIPMULADD = _register_op("CLIPMULADD_DIFF15436", Spec(
    body=minn(maxx(Src0, C0), C1) * C2 + Src1,
    reference=lambda in0, in1, s0, s1, imm2:
        (np.minimum(np.maximum(in0, s0), s1) * imm2 + in1).astype(np.float32),
))


# ---------------------------------------------------------------- schedule
def _vp_schedule():
    t = np.arange(1, T_STEPS + 1, dtype=np.float64)
    b_max, b_min = 10.0, 0.1
    alpha = np.exp(-b_min / T_STEPS - 0.5 * (b_max - b_min) * (2 * t - 1) / T_STEPS ** 2)
    betas = 1.0 - alpha
    ac = np.cumprod(1.0 - betas)
    ac_prev = np.concatenate([[1.0], ac[:-1]])
    return {
        'c1': np.sqrt(1.0 / ac).astype(np.float32),
        'c2': np.sqrt(1.0 / ac - 1.0).astype(np.float32),
        'p1': (betas * np.sqrt(ac_prev) / (1.0 - ac)).astype(np.float32),
        'p2': ((1.0 - ac_prev) * np.sqrt(1.0 - betas) / (1.0 - ac)).astype(np.float32),
        'logvar': np.log(np.clip(betas * (1.0 - ac_prev) / (1.0 - ac), 1e-20, None)).astype(np.float32),
    }


def _mish64(v):
    return v * np.tanh(np.logaddexp(0.0, v))


# ---------------------------------------------------------------- bass build
_CACHE = {}


def _build(nsteps=T_STEPS):
    if ('nc', nsteps) in _CACHE:
        return _CACHE[('nc', nsteps)]
    sched = _vp_schedule()
    c1s, c2s, p1s, p2s = sched['c1'], sched['c2'], sched['p1'], sched['p2']

    nc = bacc.Bacc("TRN2", target_bir_lowering=False, debug=False, num_devices=NCORES)
    f32 = mybir.dt.float32
    bf = mybir.dt.bfloat16

    def din(name, shape, dt=f32):
        return nc.dram_tensor(name, shape, dt, kind="ExternalInput").ap()

    d_state = din("state_t", [STATE_DIM, BPC], bf)
    d_xinit = din("x_init_t", [ACTION_DIM, BPC])
    d_noise = din("noise_t", [T_STEPS, ACTION_DIM, BPC])
    d_w1x_hi = din("w1x_hi", [KX, HIDDEN], bf)
    d_w1x_lo = din("w1x_lo", [KX, HIDDEN], bf)
    d_w2_hi = din("w2_hi", [HIDDEN, HIDDEN], bf)
    d_w3_hi = din("w3_hi", [HIDDEN, HIDDEN], bf)
    d_w4_hi = din("w4_hi", [HIDDEN, ACTION_DIM], bf)
    d_w4_lo = din("w4_lo", [HIDDEN, ACTION_DIM], bf)
    d_cont = din("cont_hl", [2, T_STEPS * HIDDEN], bf)
    d_b23 = din("b23_hl", [2, 2 * HIDDEN], bf)
    d_xb = din("xb_t", [ACTION_DIM, T_STEPS])
    d_out = nc.dram_tensor("out_t", [ACTION_DIM, BPC], f32, kind="ExternalOutput").ap()

    with tile.TileContext(nc) as tc, ExitStack() as ctx:
        wp = ctx.enter_context(tc.tile_pool(name="weights", bufs=1))
        ap_ = ctx.enter_context(tc.tile_pool(name="acts", bufs=2))
        sp = ctx.enter_context(tc.tile_pool(name="small", bufs=2))
        np_ = ctx.enter_context(tc.tile_pool(name="noise", bufs=4))
        pp = ctx.enter_context(tc.tile_pool(name="psum", bufs=2, space="PSUM"))

        def wtile(shape, dt, nm, src):
            t = wp.tile(shape, dt, tag=nm, name=nm)
            nc.gpsimd.dma_start(t, src)
            return t

        w1x_hi = wtile([KX, HIDDEN], bf, "w1x_hi", d_w1x_hi)
        w1x_lo = wtile([KX, HIDDEN], bf, "w1x_lo", d_w1x_lo)
        w2 = {}
        w3 = {}
        w4 = {}
        for nm, dhi, dst in (("w2", d_w2_hi, w2), ("w3", d_w3_hi, w3)):
            for kc in (0, 1):
                dst[("hi", kc)] = wtile([128, HIDDEN], bf, f"{nm}_hi_{kc}",
                                        dhi[kc * 128:(kc + 1) * 128, :])
        for hl, dd in (("hi", d_w4_hi), ("lo", d_w4_lo)):
            for kc in (0, 1):
                w4[(hl, kc)] = wtile([128, ACTION_DIM], bf, f"w4_{hl}_{kc}",
                                     dd[kc * 128:(kc + 1) * 128, :])
        cont = wtile([2, T_STEPS * HIDDEN], bf, "cont", d_cont)
        b23 = wtile([2, 2 * HIDDEN], bf, "b23", d_b23)
        xb = wtile([ACTION_DIM, T_STEPS], f32, "xb", d_xb)

        ones = wp.tile([2, BPC], bf, tag="ones", name="ones")
        nc.vector.memset(ones, 1.0)
        sig_bias = wp.tile([128, 1], f32, tag="sig_bias", name="sig_bias")
        nc.vector.memset(sig_bias, -D_S)

        hT = wp.tile([KX, BPC], bf, tag="hT", name="hT")
        nc.gpsimd.dma_start(hT[ACTION_DIM:KX, :], d_state)
        xT = wp.tile([ACTION_DIM, BPC], f32, tag="xT", name="xT")
        nc.gpsimd.dma_start(xT, d_xinit)

        SIG = mybir.ActivationFunctionType.Sigmoid
        MUL = mybir.AluOpType.mult
        ADD = mybir.AluOpType.add
        MAX = mybir.AluOpType.max
        MIN = mybir.AluOpType.min

        for k in range(nsteps):
            i = T_STEPS - 1 - k
            c1 = float(c1s[i]); c2 = float(c2s[i])
            p1 = float(p1s[i]); p2 = float(p2s[i])

            # bf16 view of x for the L1 matmul
            nc.vector.tensor_copy(hT[0:ACTION_DIM, :], xT)

            # noise for this step (pre-scaled by sigma on the host)
            nz = np_.tile([ACTION_DIM, BPC], f32, tag="nz", name="nz")
            nc.sync.dma_start(nz, d_noise[k])

            # early elementwise pieces (only depend on x_k and noise)
            s2 = sp.tile([ACTION_DIM, BPC], f32, tag="s2", name="s2")
            nc.vector.scalar_tensor_tensor(s2, xT, p2, nz, MUL, ADD)

            # ---- the 3 hidden layers ----
            hprev = None
            for L, (wd, bias_off) in enumerate((
                    (None, None), (w2, 0), (w3, HIDDEN))):
                z = pp.tile([128, 2 * BPC], mybir.dt.float32, tag=f"z{L}", name=f"z{L}")
                for mc in (0, 1):
                    zslice = z[:, mc * BPC:(mc + 1) * BPC]
                    if L == 0:
                        off = i * HIDDEN + mc * 128
                        nc.tensor.matmul(zslice, cont[0:2, off:off + 128], ones, start=True, stop=False)
                        nc.tensor.matmul(zslice, w1x_hi[:, mc * 128:(mc + 1) * 128], hT, start=False, stop=False)
                        nc.tensor.matmul(zslice, w1x_lo[:, mc * 128:(mc + 1) * 128], hT, start=False, stop=True)
                    else:
                        off = bias_off + mc * 128
                        nc.tensor.matmul(zslice, b23[0:2, off:off + 128], ones, start=True, stop=False)
                        for kc in (0, 1):
                            rhs = hprev[:, kc * BPC:(kc + 1) * BPC]
                            nc.tensor.matmul(zslice, wd[("hi", kc)][:, mc * 128:(mc + 1) * 128], rhs,
                                             start=False, stop=(kc == 1))
                # sigmoid pass: s = sigmoid(-(A_S*z + D_S))
                s = ap_.tile([128, 2 * BPC], mybir.dt.float32, tag="s", name="s")
                nc.scalar.activation(s, z, SIG, bias=sig_bias, scale=-A_S)
                # custom completion: h = z*QUAD(t)*CUBIC(t), t = s^2
                wA = ap_.tile([128, 2 * BPC], mybir.dt.float32, tag="wA", name="wA")
                nc.vector._custom_dve(MISH_A, out=wA, in0=s, in1=z, s0=QA, s1=QB, imm2=QC)
                h = ap_.tile([128, 2 * BPC], bf, tag=f"h{L}", name=f"h{L}")
                nc.vector._custom_dve(MISH_B, out=h, in0=s, in1=wA, s0=MA, s1=MB, imm2=MC)
                hprev = h

            # ---- L4: eps psum [16, BPC] ----
            z4 = pp.tile([ACTION_DIM, BPC], mybir.dt.float32, tag="z4", name="z4")
            nc.tensor.matmul(z4, w4[("hi", 0)], hprev[:, 0:BPC], start=True, stop=False)
            nc.tensor.matmul(z4, w4[("lo", 0)], hprev[:, 0:BPC], start=False, stop=False)
            nc.tensor.matmul(z4, w4[("hi", 1)], hprev[:, BPC:2 * BPC], start=False, stop=False)
            nc.tensor.matmul(z4, w4[("lo", 1)], hprev[:, BPC:2 * BPC], start=False, stop=True)

            # ---- x update ----
            pre = sp.tile([ACTION_DIM, BPC], f32, tag="pre", name="pre")
            nc.vector._custom_dve(PREOP, out=pre, in0=z4, in1=xT,
                                  s0=xb[:, i:i + 1], s1=c1, imm2=-c2)
            # xT <- clip(pre, -1, 1)*p1 + s2
            nc.vector._custom_dve(CLIPMULADD, out=xT, in0=pre, in1=s2,
                                  s0=-1.0, s1=1.0, imm2=p1)

        out_f = sp.tile([ACTION_DIM, BPC], f32, tag="out_f", name="out_f")
        nc.vector.tensor_scalar(out_f, xT, -1.0, 1.0, MAX, MIN)
        nc.sync.dma_start(d_out, out_f)

    nc.compile()
    _CACHE[('nc', nsteps)] = nc
    return nc


# ---------------------------------------------------------------- host side
def _host_prep(inputs):
    sched = _vp_schedule()
    f64 = np.float64

    W1 = np.asarray(inputs['W1'], np.float32)
    b1 = np.asarray(inputs['b1'], np.float32)
    W2 = np.asarray(inputs['W2'], np.float32)
    b2 = np.asarray(inputs['b2'], np.float32)
    W3 = np.asarray(inputs['W3'], np.float32)
    b3 = np.asarray(inputs['b3'], np.float32)
    W4 = np.asarray(inputs['W4'], np.float32)
    b4 = np.asarray(inputs['b4'], np.float32)

    # time-embedding MLP for all 100 timesteps (host, float64)
    half = TIME_DIM // 2
    freqs = np.exp(np.arange(half, dtype=f64) * (-math.log(10000.0) / (half - 1)))
    ivals = np.arange(T_STEPS, dtype=f64)
    ang = ivals[:, None] * freqs[None, :]
    emb = np.concatenate([np.sin(ang), np.cos(ang)], axis=1)
    t1 = _mish64(emb @ np.asarray(inputs['time_W1'], f64) + np.asarray(inputs['time_b1'], f64))
    temb = t1 @ np.asarray(inputs['time_W2'], f64) + np.asarray(inputs['time_b2'], f64)

    # beta-folded biases
    b2e = b2.astype(f64) + BETA * W2.astype(f64).sum(axis=0)
    b3e = b3.astype(f64) + BETA * W3.astype(f64).sum(axis=0)
    b4e = b4.astype(f64) + BETA * W4.astype(f64).sum(axis=0)

    # contrib[i] = temb[i] @ W1[16:48] + b1   -> flat [1, 100*256]
    contrib = (temb @ W1[16:48].astype(f64) + b1.astype(f64))  # [100, 256]

    def hilo(v):
        v32 = np.asarray(v, np.float32)
        hi = v32.astype(BF16)
        lo = (v32 - hi.astype(np.float32)).astype(BF16)
        return hi, lo

    c_hi, c_lo = hilo(contrib.reshape(1, -1))
    cont_hl = np.concatenate([c_hi, c_lo], axis=0)
    bb_hi, bb_lo = hilo(np.concatenate([b2e, b3e]).reshape(1, -1))
    b23_hl = np.concatenate([bb_hi, bb_lo], axis=0)
    w1x = np.concatenate([W1[0:16], W1[48:112]], axis=0)
    w1x_hi, w1x_lo = hilo(w1x)
    w2_hi = np.asarray(W2, np.float32).astype(BF16)
    w3_hi = np.asarray(W3, np.float32).astype(BF16)
    w4_hi, w4_lo = hilo(W4)

    # x-update tables
    xb = (-sched['c2'].astype(f64)[None, :] * b4e[:, None]).astype(np.float32)  # [16, 100]

    # per-step noise scaling (fp32, matching the reference ops)
    sig = np.exp(0.5 * sched['logvar']).astype(np.float32)  # [100] by timestep i
    ik = (T_STEPS - 1 - np.arange(T_STEPS))                 # timestep for step k
    scale = sig[ik] * (ik != 0).astype(np.float32)          # [100]
    noise = np.asarray(inputs['noise'], np.float32)
    noise_scaled = noise * scale[:, None, None]

    state = np.asarray(inputs['state'], np.float32)
    x_init = np.asarray(inputs['x_init'], np.float32)

    shared = dict(
        w1x_hi=w1x_hi, w1x_lo=w1x_lo, w2_hi=w2_hi,
        w3_hi=w3_hi, w4_hi=w4_hi, w4_lo=w4_lo,
        cont_hl=cont_hl, b23_hl=b23_hl,
        xb_t=xb,
    )
    in_maps = []
    for c in range(NCORES):
        sl = slice(c * BPC, (c + 1) * BPC)
        m = dict(shared)
        m['state_t'] = np.ascontiguousarray(state[sl].T).astype(BF16)
        m['x_init_t'] = np.ascontiguousarray(x_init[sl].T)
        m['noise_t'] = np.ascontiguousarray(noise_scaled[:, sl, :].transpose(0, 2, 1))
        in_maps.append(m)
    return in_maps


def run(inputs, trace=False, nsteps=T_STEPS):
    nc = _build(nsteps)
    in_maps = _host_prep(inputs)
    res = bass_utils.run_bass_kernel_spmd(
        nc, in_maps, core_ids=list(range(NCORES)), trace=trace)
    out = np.empty((BATCH, ACTION_DIM), np.float32)
    for c in range(NCORES):
        out[c * BPC:(c + 1) * BPC] = res.results[c]['out_t'].T
    return out, res


def kernel(**inputs) -> np.ndarray:
    out, _ = run(inputs, trace=False)
    return out


# revision 12
# speedup vs baseline: 1.5031x; 1.0709x over previous
"""Trainium2 Bass kernel for nn_Diffusion_15436112462451.

Strategy: pure data parallelism over the batch (2048 -> 8 cores x 256),
feature-major activations on-chip, fully unrolled 100-step loop.

Per step (per core):
  - 36 bf16 matmuls on PE: split-precision weights (W = Whi + Wlo in bf16,
    two matmuls accumulating in fp32 PSUM) + rank-1 bias matmuls.
  - 3 sigmoid passes on ScalarE (one per hidden layer).
  - 2 fused custom-DVE passes per layer evaluate the exact-mish rational
    completion  mish(z) ~= z * QUAD(t) * CUBIC(t) + beta,  t = sigmoid(-az-d)^2
    (degree-5 minimax fit of tanh(softplus), max err 6.3e-5; beta is folded
    into the next layer's bias on the host).
  - The denoising x-update runs on small [16,256] DVE ops with per-step
    schedule scalars baked in as immediates.

The time-embedding MLP is batch-independent (the timestep is a scalar per
step), so its contribution is precomputed on the host into a [100,256] bias
table and injected via rank-1 bias matmuls.
"""
import sys
import math
import re
import numpy as np

for _p in ('/opt/trn_rl_repo', '/root/.axon_site/_ro/trn_rl_repo'):
    if _p not in sys.path:
        sys.path.insert(0, _p)

import ml_dtypes
from contextlib import ExitStack
import concourse.bass as bass
from concourse import bacc
from concourse import mybir, tile, bass_utils, dve_ops
from concourse.dve_spec import Spec, Src0, Src1, C0, C1, C2, sq, maxx, minn

BF16 = ml_dtypes.bfloat16
NCORES = 8
BATCH = 2048
BPC = BATCH // NCORES          # 256 batch rows per core
T_STEPS = 100
STATE_DIM, ACTION_DIM, HIDDEN, TIME_DIM = 64, 16, 256, 32
KX = ACTION_DIM + STATE_DIM    # 80 rows of W1 used for [x; state]

# --- activation fit constants (deg-5 sigma-poly factorization) ---
A_S = 0.9990298806699722
D_S = -0.0005000143935776705
BETA = 4.708088756431602e-05
QA, QB, QC = -0.21302398380145082, 0.6455208072356895, -0.6201860532189531
MA, MB, MC = -0.9194163848641597, 1.5334239721923986, -1.6124382654378613


# ---------------------------------------------------------------- custom ops
def _register_op(name, spec):
    for op in dve_ops.OPS:
        if op.name == name:
            return op
    op = dve_ops.DveOp(name, spec, False, uops_sha={"v3": "?", "v4": "?"})
    dve_ops.OPS.append(op)
    dve_ops.CUSTOM_DVE_SPECS[name] = spec
    dve_ops._SUB_OPCODE_FOR_NAME[name] = (
        dve_ops._CUSTOM_DVE_ROW_BASE + len(dve_ops.OPS) - 1)
    for ver in ("v3", "v4"):
        try:
            op.compile(ver)
        except ValueError as e:
            op.uops_sha[ver] = re.search(
                r'uops_sha\["' + ver + r'"\]="([0-9a-f]+)"', str(e)).group(1)
        op.compile(ver)
    return op


_t = sq(Src0)
MISH_A = _register_op("MISH_A_DIFF15436", Spec(
    body=Src1 * ((_t * C0 + C1) * _t + C2),
    reference=lambda in0, in1, s0, s1, imm2:
        (in1 * ((s0 * in0.astype(np.float64) ** 2 + s1) * in0.astype(np.float64) ** 2 + imm2)).astype(np.float32),
))
_t2 = sq(Src0)
MISH_B = _register_op("MISH_B_DIFF15436", Spec(
    body=Src1 * ((((_t2 + C0) * _t2 + C1) * _t2) + C2),
    reference=lambda in0, in1, s0, s1, imm2:
        (in1 * ((((in0.astype(np.float64) ** 2 + s0) * in0.astype(np.float64) ** 2 + s1) * in0.astype(np.float64) ** 2) + imm2)).astype(np.float32),
))
PREOP = _register_op("PREOP_DIFF15436", Spec(
    body=Src0 * C2 + Src1 * C1 + C0,
    reference=lambda in0, in1, s0, s1, imm2:
        (in0 * imm2 + in1 * s1 + s0).astype(np.float32),
))
CLIPMULADD = _register_op("CLIPMULADD_DIFF15436", Spec(
    body=minn(maxx(Src0, C0), C1) * C2 + Src1,
    reference=lambda in0, in1, s0, s1, imm2:
        (np.minimum(np.maximum(in0, s0), s1) * imm2 + in1).astype(np.float32),
))


# ---------------------------------------------------------------- schedule
def _vp_schedule():
    t = np.arange(1, T_STEPS + 1, dtype=np.float64)
    b_max, b_min = 10.0, 0.1
    alpha = np.exp(-b_min / T_STEPS - 0.5 * (b_max - b_min) * (2 * t - 1) / T_STEPS ** 2)
    betas = 1.0 - alpha
    ac = np.cumprod(1.0 - betas)
    ac_prev = np.concatenate([[1.0], ac[:-1]])
    return {
        'c1': np.sqrt(1.0 / ac).astype(np.float32),
        'c2': np.sqrt(1.0 / ac - 1.0).astype(np.float32),
        'p1': (betas * np.sqrt(ac_prev) / (1.0 - ac)).astype(np.float32),
        'p2': ((1.0 - ac_prev) * np.sqrt(1.0 - betas) / (1.0 - ac)).astype(np.float32),
        'logvar': np.log(np.clip(betas * (1.0 - ac_prev) / (1.0 - ac), 1e-20, None)).astype(np.float32),
    }


def _mish64(v):
    return v * np.tanh(np.logaddexp(0.0, v))


# ---------------------------------------------------------------- bass build
_CACHE = {}


def _build(nsteps=T_STEPS):
    if ('nc', nsteps) in _CACHE:
        return _CACHE[('nc', nsteps)]
    sched = _vp_schedule()
    c1s, c2s, p1s, p2s = sched['c1'], sched['c2'], sched['p1'], sched['p2']

    nc = bacc.Bacc("TRN2", target_bir_lowering=False, debug=False, num_devices=NCORES)
    f32 = mybir.dt.float32
    bf = mybir.dt.bfloat16

    def din(name, shape, dt=f32):
        return nc.dram_tensor(name, shape, dt, kind="ExternalInput").ap()

    d_state = din("state_t", [STATE_DIM, BPC], bf)
    d_xinit = din("x_init_t", [ACTION_DIM, BPC])
    d_noise = din("noise_t", [T_STEPS, ACTION_DIM, BPC])
    d_w1x_hi = din("w1x_hi", [KX, HIDDEN], bf)
    d_w1x_lo = din("w1x_lo", [KX, HIDDEN], bf)
    d_w2_hi = din("w2_hi", [HIDDEN, HIDDEN], bf)
    d_w3_hi = din("w3_hi", [HIDDEN, HIDDEN], bf)
    d_w4_hi = din("w4_hi", [HIDDEN, ACTION_DIM], bf)
    d_w4_lo = din("w4_lo", [HIDDEN, ACTION_DIM], bf)
    d_cont = din("cont_hl", [4, T_STEPS * 128], bf)
    d_b23 = din("b23_hl", [4, 2 * 128], bf)
    d_mask = din("mask4", [4, 2 * BPC], bf)
    d_xb = din("xb_t", [ACTION_DIM, T_STEPS])
    d_out = nc.dram_tensor("out_t", [ACTION_DIM, BPC], f32, kind="ExternalOutput").ap()

    with tile.TileContext(nc) as tc, ExitStack() as ctx:
        wp = ctx.enter_context(tc.tile_pool(name="weights", bufs=1))
        ap_ = ctx.enter_context(tc.tile_pool(name="acts", bufs=2))
        sp = ctx.enter_context(tc.tile_pool(name="small", bufs=2))
        np_ = ctx.enter_context(tc.tile_pool(name="noise", bufs=4))
        pp = ctx.enter_context(tc.tile_pool(name="psum", bufs=2, space="PSUM"))

        def wtile(shape, dt, nm, src):
            t = wp.tile(shape, dt, tag=nm, name=nm)
            nc.gpsimd.dma_start(t, src)
            return t

        w1x_hi = wtile([KX, HIDDEN], bf, "w1x_hi", d_w1x_hi)
        w1x_lo = wtile([KX, HIDDEN], bf, "w1x_lo", d_w1x_lo)
        w2 = {}
        w3 = {}
        w4 = {}
        for nm, dhi, dst in (("w2", d_w2_hi, w2), ("w3", d_w3_hi, w3)):
            for kc in (0, 1):
                dst[("hi", kc)] = wtile([128, HIDDEN], bf, f"{nm}_hi_{kc}",
                                        dhi[kc * 128:(kc + 1) * 128, :])
        for hl, dd in (("hi", d_w4_hi), ("lo", d_w4_lo)):
            for kc in (0, 1):
                w4[(hl, kc)] = wtile([128, ACTION_DIM], bf, f"w4_{hl}_{kc}",
                                     dd[kc * 128:(kc + 1) * 128, :])
        cont = wtile([4, T_STEPS * 128], bf, "cont", d_cont)
        b23 = wtile([4, 2 * 128], bf, "b23", d_b23)
        mask4 = wtile([4, 2 * BPC], bf, "mask4", d_mask)
        noise_sb = wp.tile([ACTION_DIM, T_STEPS * BPC], f32, tag="noise_sb", name="noise_sb")
        nc.gpsimd.dma_start(
            noise_sb.rearrange("p (k c) -> p k c", k=T_STEPS),
            d_noise.rearrange("k p c -> p k c"))
        xb = wtile([ACTION_DIM, T_STEPS], f32, "xb", d_xb)

        sig_bias = wp.tile([128, 1], f32, tag="sig_bias", name="sig_bias")
        nc.vector.memset(sig_bias, -D_S)

        hT = wp.tile([KX, BPC], bf, tag="hT", name="hT")
        nc.gpsimd.dma_start(hT[ACTION_DIM:KX, :], d_state)
        xT = wp.tile([ACTION_DIM, BPC], f32, tag="xT", name="xT")
        nc.gpsimd.dma_start(xT, d_xinit)

        SIG = mybir.ActivationFunctionType.Sigmoid
        MUL = mybir.AluOpType.mult
        ADD = mybir.AluOpType.add
        MAX = mybir.AluOpType.max
        MIN = mybir.AluOpType.min

        for k in range(nsteps):
            i = T_STEPS - 1 - k
            c1 = float(c1s[i]); c2 = float(c2s[i])
            p1 = float(p1s[i]); p2 = float(p2s[i])

            # bf16 view of x for the L1 matmul
            nc.vector.tensor_copy(hT[0:ACTION_DIM, :], xT)

            # early elementwise pieces (only depend on x_k and preloaded noise)
            nz = noise_sb[:, k * BPC:(k + 1) * BPC]
            s2 = sp.tile([ACTION_DIM, BPC], f32, tag="s2", name="s2")
            nc.vector.scalar_tensor_tensor(s2, xT, p2, nz, MUL, ADD)

            # ---- the 3 hidden layers ----
            hprev = None
            for L, (wd, bias_off) in enumerate((
                    (None, None), (w2, 0), (w3, HIDDEN))):
                z = pp.tile([128, 2 * BPC], mybir.dt.float32, tag=f"z{L}", name=f"z{L}")
                if L == 0:
                    boff = i * 128
                    nc.tensor.matmul(z, cont[0:4, boff:boff + 128], mask4, start=True, stop=False)
                else:
                    boff = (bias_off // HIDDEN) * 128
                    nc.tensor.matmul(z, b23[0:4, boff:boff + 128], mask4, start=True, stop=False)
                for mc in (0, 1):
                    zslice = z[:, mc * BPC:(mc + 1) * BPC]
                    if L == 0:
                        nc.tensor.matmul(zslice, w1x_hi[:, mc * 128:(mc + 1) * 128], hT, start=False, stop=False)
                        nc.tensor.matmul(zslice, w1x_lo[:, mc * 128:(mc + 1) * 128], hT, start=False, stop=True)
                    else:
                        for kc in (0, 1):
                            rhs = hprev[:, kc * BPC:(kc + 1) * BPC]
                            nc.tensor.matmul(zslice, wd[("hi", kc)][:, mc * 128:(mc + 1) * 128], rhs,
                                             start=False, stop=(kc == 1))
                # sigmoid pass: s = sigmoid(-(A_S*z + D_S))
                s = ap_.tile([128, 2 * BPC], mybir.dt.float32, tag="s", name="s")
                nc.scalar.activation(s, z, SIG, bias=sig_bias, scale=-A_S)
                # custom completion: h = z*QUAD(t)*CUBIC(t), t = s^2
                wA = ap_.tile([128, 2 * BPC], mybir.dt.float32, tag="wA", name="wA")
                nc.vector._custom_dve(MISH_A, out=wA, in0=s, in1=z, s0=QA, s1=QB, imm2=QC)
                h = ap_.tile([128, 2 * BPC], bf, tag=f"h{L}", name=f"h{L}")
                nc.vector._custom_dve(MISH_B, out=h, in0=s, in1=wA, s0=MA, s1=MB, imm2=MC)
                hprev = h

            # ---- L4: eps psum [16, BPC] ----
            z4 = pp.tile([ACTION_DIM, BPC], mybir.dt.float32, tag="z4", name="z4")
            nc.tensor.matmul(z4, w4[("hi", 0)], hprev[:, 0:BPC], start=True, stop=False)
            nc.tensor.matmul(z4, w4[("lo", 0)], hprev[:, 0:BPC], start=False, stop=False)
            nc.tensor.matmul(z4, w4[("hi", 1)], hprev[:, BPC:2 * BPC], start=False, stop=False)
            nc.tensor.matmul(z4, w4[("lo", 1)], hprev[:, BPC:2 * BPC], start=False, stop=True)

            # ---- x update ----
            pre = sp.tile([ACTION_DIM, BPC], f32, tag="pre", name="pre")
            nc.vector._custom_dve(PREOP, out=pre, in0=z4, in1=xT,
                                  s0=xb[:, i:i + 1], s1=c1, imm2=-c2)
            # xT <- clip(pre, -1, 1)*p1 + s2
            nc.vector._custom_dve(CLIPMULADD, out=xT, in0=pre, in1=s2,
                                  s0=-1.0, s1=1.0, imm2=p1)

        out_f = sp.tile([ACTION_DIM, BPC], f32, tag="out_f", name="out_f")
        nc.vector.tensor_scalar(out_f, xT, -1.0, 1.0, MAX, MIN)
        nc.sync.dma_start(d_out, out_f)

    nc.compile()
    _CACHE[('nc', nsteps)] = nc
    return nc


# ---------------------------------------------------------------- host side
def _host_prep(inputs):
    sched = _vp_schedule()
    f64 = np.float64

    W1 = np.asarray(inputs['W1'], np.float32)
    b1 = np.asarray(inputs['b1'], np.float32)
    W2 = np.asarray(inputs['W2'], np.float32)
    b2 = np.asarray(inputs['b2'], np.float32)
    W3 = np.asarray(inputs['W3'], np.float32)
    b3 = np.asarray(inputs['b3'], np.float32)
    W4 = np.asarray(inputs['W4'], np.float32)
    b4 = np.asarray(inputs['b4'], np.float32)

    # time-embedding MLP for all 100 timesteps (host, float64)
    half = TIME_DIM // 2
    freqs = np.exp(np.arange(half, dtype=f64) * (-math.log(10000.0) / (half - 1)))
    ivals = np.arange(T_STEPS, dtype=f64)
    ang = ivals[:, None] * freqs[None, :]
    emb = np.concatenate([np.sin(ang), np.cos(ang)], axis=1)
    t1 = _mish64(emb @ np.asarray(inputs['time_W1'], f64) + np.asarray(inputs['time_b1'], f64))
    temb = t1 @ np.asarray(inputs['time_W2'], f64) + np.asarray(inputs['time_b2'], f64)

    # beta-folded biases
    b2e = b2.astype(f64) + BETA * W2.astype(f64).sum(axis=0)
    b3e = b3.astype(f64) + BETA * W3.astype(f64).sum(axis=0)
    b4e = b4.astype(f64) + BETA * W4.astype(f64).sum(axis=0)

    # contrib[i] = temb[i] @ W1[16:48] + b1   -> flat [1, 100*256]
    contrib = (temb @ W1[16:48].astype(f64) + b1.astype(f64))  # [100, 256]

    def hilo(v):
        v32 = np.asarray(v, np.float32)
        hi = v32.astype(BF16)
        lo = (v32 - hi.astype(np.float32)).astype(BF16)
        return hi, lo

    def pack4(v2d):
        # v2d [G, 256] -> [4, G*128]: rows (hi_a, lo_a, hi_b, lo_b)
        hi, lo = hilo(v2d)
        hi = hi.astype(np.float32); lo = lo.astype(np.float32)
        out = np.stack([hi[:, :128], lo[:, :128], hi[:, 128:], lo[:, 128:]], axis=0)
        return out.reshape(4, -1).astype(BF16)
    cont_hl = pack4(contrib.astype(np.float32))
    b23_hl = pack4(np.stack([b2e, b3e]).astype(np.float32))
    mask4 = np.zeros((4, 2 * BPC), np.float32)
    mask4[0:2, :BPC] = 1.0
    mask4[2:4, BPC:] = 1.0
    mask4 = mask4.astype(BF16)
    w1x = np.concatenate([W1[0:16], W1[48:112]], axis=0)
    w1x_hi, w1x_lo = hilo(w1x)
    w2_hi = np.asarray(W2, np.float32).astype(BF16)
    w3_hi = np.asarray(W3, np.float32).astype(BF16)
    w4_hi, w4_lo = hilo(W4)

    # x-update tables
    xb = (-sched['c2'].astype(f64)[None, :] * b4e[:, None]).astype(np.float32)  # [16, 100]

    # per-step noise scaling (fp32, matching the reference ops)
    sig = np.exp(0.5 * sched['logvar']).astype(np.float32)  # [100] by timestep i
    ik = (T_STEPS - 1 - np.arange(T_STEPS))                 # timestep for step k
    scale = sig[ik] * (ik != 0).astype(np.float32)          # [100]
    noise = np.asarray(inputs['noise'], np.float32)
    noise_scaled = noise * scale[:, None, None]

    state = np.asarray(inputs['state'], np.float32)
    x_init = np.asarray(inputs['x_init'], np.float32)

    shared = dict(
        w1x_hi=w1x_hi, w1x_lo=w1x_lo, w2_hi=w2_hi,
        w3_hi=w3_hi, w4_hi=w4_hi, w4_lo=w4_lo,
        cont_hl=cont_hl, b23_hl=b23_hl, mask4=mask4,
        xb_t=xb,
    )
    in_maps = []
    for c in range(NCORES):
        sl = slice(c * BPC, (c + 1) * BPC)
        m = dict(shared)
        m['state_t'] = np.ascontiguousarray(state[sl].T).astype(BF16)
        m['x_init_t'] = np.ascontiguousarray(x_init[sl].T)
        m['noise_t'] = np.ascontiguousarray(noise_scaled[:, sl, :].transpose(0, 2, 1))
        in_maps.append(m)
    return in_maps


def run(inputs, trace=False, nsteps=T_STEPS):
    nc = _build(nsteps)
    in_maps = _host_prep(inputs)
    res = bass_utils.run_bass_kernel_spmd(
        nc, in_maps, core_ids=list(range(NCORES)), trace=trace)
    out = np.empty((BATCH, ACTION_DIM), np.float32)
    for c in range(NCORES):
        out[c * BPC:(c + 1) * BPC] = res.results[c]['out_t'].T
    return out, res


def kernel(**inputs) -> np.ndarray:
    out, _ = run(inputs, trace=False)
    return out


# revision 13
# speedup vs baseline: 1.7757x; 1.1813x over previous
"""Trainium2 Bass kernel for nn_Diffusion_15436112462451.

Strategy: pure data parallelism over the batch (2048 -> 8 cores x 256),
feature-major activations on-chip, fully unrolled 100-step loop.

Per step (per core):
  - 36 bf16 matmuls on PE: split-precision weights (W = Whi + Wlo in bf16,
    two matmuls accumulating in fp32 PSUM) + rank-1 bias matmuls.
  - 3 sigmoid passes on ScalarE (one per hidden layer).
  - 2 fused custom-DVE passes per layer evaluate the exact-mish rational
    completion  mish(z) ~= z * QUAD(t) * CUBIC(t) + beta,  t = sigmoid(-az-d)^2
    (degree-5 minimax fit of tanh(softplus), max err 6.3e-5; beta is folded
    into the next layer's bias on the host).
  - The denoising x-update runs on small [16,256] DVE ops with per-step
    schedule scalars baked in as immediates.

The time-embedding MLP is batch-independent (the timestep is a scalar per
step), so its contribution is precomputed on the host into a [100,256] bias
table and injected via rank-1 bias matmuls.
"""
import sys
import math
import re
import numpy as np

for _p in ('/opt/trn_rl_repo', '/root/.axon_site/_ro/trn_rl_repo'):
    if _p not in sys.path:
        sys.path.insert(0, _p)

import ml_dtypes
from contextlib import ExitStack
import concourse.bass as bass
from concourse import bacc
from concourse import mybir, tile, bass_utils, dve_ops
from concourse.dve_spec import Spec, Src0, Src1, C0, C1, C2, sq, maxx, minn

BF16 = ml_dtypes.bfloat16
NCORES = 8
BATCH = 2048
BPC = BATCH // NCORES          # 256 batch rows per core
T_STEPS = 100
STATE_DIM, ACTION_DIM, HIDDEN, TIME_DIM = 64, 16, 256, 32
KX = ACTION_DIM + STATE_DIM    # 80 rows of W1 used for [x; state]

# --- activation fit constants (deg-5 sigma-poly factorization) ---
A_S = 0.9990298806699722
D_S = -0.0005000143935776705
BETA = 4.708088756431602e-05
QA, QB, QC = -0.21302398380145082, 0.6455208072356895, -0.6201860532189531
MA, MB, MC = -0.9194163848641597, 1.5334239721923986, -1.6124382654378613


# ---------------------------------------------------------------- custom ops
def _register_op(name, spec):
    for op in dve_ops.OPS:
        if op.name == name:
            return op
    op = dve_ops.DveOp(name, spec, False, uops_sha={"v3": "?", "v4": "?"})
    dve_ops.OPS.append(op)
    dve_ops.CUSTOM_DVE_SPECS[name] = spec
    dve_ops._SUB_OPCODE_FOR_NAME[name] = (
        dve_ops._CUSTOM_DVE_ROW_BASE + len(dve_ops.OPS) - 1)
    for ver in ("v3", "v4"):
        try:
            op.compile(ver)
        except ValueError as e:
            op.uops_sha[ver] = re.search(
                r'uops_sha\["' + ver + r'"\]="([0-9a-f]+)"', str(e)).group(1)
        op.compile(ver)
    return op


_t = sq(Src0)
MISH_A = _register_op("MISH_A_DIFF15436", Spec(
    body=Src1 * ((_t * C0 + C1) * _t + C2),
    reference=lambda in0, in1, s0, s1, imm2:
        (in1 * ((s0 * in0.astype(np.float64) ** 2 + s1) * in0.astype(np.float64) ** 2 + imm2)).astype(np.float32),
))
_t2 = sq(Src0)
MISH_B = _register_op("MISH_B_DIFF15436", Spec(
    body=Src1 * ((((_t2 + C0) * _t2 + C1) * _t2) + C2),
    reference=lambda in0, in1, s0, s1, imm2:
        (in1 * ((((in0.astype(np.float64) ** 2 + s0) * in0.astype(np.float64) ** 2 + s1) * in0.astype(np.float64) ** 2) + imm2)).astype(np.float32),
))
PREOP = _register_op("PREOP_DIFF15436", Spec(
    body=Src0 * C2 + Src1 * C1 + C0,
    reference=lambda in0, in1, s0, s1, imm2:
        (in0 * imm2 + in1 * s1 + s0).astype(np.float32),
))
CLIPMULADD = _register_op("CLIPMULADD_DIFF15436", Spec(
    body=minn(maxx(Src0, C0), C1) * C2 + Src1,
    reference=lambda in0, in1, s0, s1, imm2:
        (np.minimum(np.maximum(in0, s0), s1) * imm2 + in1).astype(np.float32),
))


# ---------------------------------------------------------------- schedule
def _vp_schedule():
    t = np.arange(1, T_STEPS + 1, dtype=np.float64)
    b_max, b_min = 10.0, 0.1
    alpha = np.exp(-b_min / T_STEPS - 0.5 * (b_max - b_min) * (2 * t - 1) / T_STEPS ** 2)
    betas = 1.0 - alpha
    ac = np.cumprod(1.0 - betas)
    ac_prev = np.concatenate([[1.0], ac[:-1]])
    return {
        'c1': np.sqrt(1.0 / ac).astype(np.float32),
        'c2': np.sqrt(1.0 / ac - 1.0).astype(np.float32),
        'p1': (betas * np.sqrt(ac_prev) / (1.0 - ac)).astype(np.float32),
        'p2': ((1.0 - ac_prev) * np.sqrt(1.0 - betas) / (1.0 - ac)).astype(np.float32),
        'logvar': np.log(np.clip(betas * (1.0 - ac_prev) / (1.0 - ac), 1e-20, None)).astype(np.float32),
    }


def _mish64(v):
    return v * np.tanh(np.logaddexp(0.0, v))


# ---------------------------------------------------------------- bass build
_CACHE = {}


def _build(nsteps=T_STEPS):
    if ('nc', nsteps) in _CACHE:
        return _CACHE[('nc', nsteps)]
    sched = _vp_schedule()
    c1s, c2s, p1s, p2s = sched['c1'], sched['c2'], sched['p1'], sched['p2']

    nc = bacc.Bacc("TRN2", target_bir_lowering=False, debug=False, num_devices=NCORES)
    f32 = mybir.dt.float32
    bf = mybir.dt.bfloat16

    def din(name, shape, dt=f32):
        return nc.dram_tensor(name, shape, dt, kind="ExternalInput").ap()

    d_state = din("state_t", [STATE_DIM, BPC], bf)
    d_xinit = din("x_init_t", [ACTION_DIM, BPC])
    d_noise = din("noise_t", [T_STEPS, ACTION_DIM, BPC])
    d_w1x_hi = din("w1x_hi", [KX, HIDDEN], bf)
    d_w1x_lo = din("w1x_lo", [KX, HIDDEN], bf)
    d_w2_hi = din("w2_hi", [HIDDEN, HIDDEN], bf)
    d_w3_hi = din("w3_hi", [HIDDEN, HIDDEN], bf)
    d_w4_hi = din("w4_hi", [HIDDEN, ACTION_DIM], bf)
    d_cont = din("cont_hl", [4, T_STEPS * 128], bf)
    d_b23 = din("b23_hl", [4, 2 * 128], bf)
    d_mask = din("mask4", [4, 2 * BPC], bf)
    d_xb = din("xb_t", [ACTION_DIM, T_STEPS])
    d_out = nc.dram_tensor("out_t", [ACTION_DIM, BPC], f32, kind="ExternalOutput").ap()

    with tile.TileContext(nc) as tc, ExitStack() as ctx:
        wp = ctx.enter_context(tc.tile_pool(name="weights", bufs=1))
        ap_ = ctx.enter_context(tc.tile_pool(name="acts", bufs=2))
        sp = ctx.enter_context(tc.tile_pool(name="small", bufs=2))
        np_ = ctx.enter_context(tc.tile_pool(name="noise", bufs=4))
        pp = ctx.enter_context(tc.tile_pool(name="psum", bufs=2, space="PSUM"))

        def wtile(shape, dt, nm, src):
            t = wp.tile(shape, dt, tag=nm, name=nm)
            nc.gpsimd.dma_start(t, src)
            return t

        w1x_hi = wtile([KX, HIDDEN], bf, "w1x_hi", d_w1x_hi)
        w1x_lo = wtile([KX, HIDDEN], bf, "w1x_lo", d_w1x_lo)
        w2 = {}
        w3 = {}
        w4 = {}
        for nm, dhi, dst in (("w2", d_w2_hi, w2), ("w3", d_w3_hi, w3)):
            for kc in (0, 1):
                dst[("hi", kc)] = wtile([128, HIDDEN], bf, f"{nm}_hi_{kc}",
                                        dhi[kc * 128:(kc + 1) * 128, :])
        for kc in (0, 1):
            w4[("hi", kc)] = wtile([128, ACTION_DIM], bf, f"w4_hi_{kc}",
                                   d_w4_hi[kc * 128:(kc + 1) * 128, :])
        cont = wtile([4, T_STEPS * 128], bf, "cont", d_cont)
        b23 = wtile([4, 2 * 128], bf, "b23", d_b23)
        mask4 = wtile([4, 2 * BPC], bf, "mask4", d_mask)
        noise_sb = wp.tile([ACTION_DIM, T_STEPS * BPC], f32, tag="noise_sb", name="noise_sb")
        nc.gpsimd.dma_start(
            noise_sb.rearrange("p (k c) -> p k c", k=T_STEPS),
            d_noise.rearrange("k p c -> p k c"))
        xb = wtile([ACTION_DIM, T_STEPS], f32, "xb", d_xb)

        sig_bias = wp.tile([128, 1], f32, tag="sig_bias", name="sig_bias")
        nc.vector.memset(sig_bias, -D_S)

        hT = wp.tile([KX, BPC], bf, tag="hT", name="hT")
        nc.gpsimd.dma_start(hT[ACTION_DIM:KX, :], d_state)
        xT = wp.tile([ACTION_DIM, BPC], f32, tag="xT", name="xT")
        nc.gpsimd.dma_start(xT, d_xinit)

        SIG = mybir.ActivationFunctionType.Sigmoid
        MUL = mybir.AluOpType.mult
        ADD = mybir.AluOpType.add
        MAX = mybir.AluOpType.max
        MIN = mybir.AluOpType.min

        for k in range(nsteps):
            i = T_STEPS - 1 - k
            c1 = float(c1s[i]); c2 = float(c2s[i])
            p1 = float(p1s[i]); p2 = float(p2s[i])

            # bf16 view of x for the L1 matmul
            nc.vector.tensor_copy(hT[0:ACTION_DIM, :], xT)

            # early elementwise pieces (only depend on x_k and preloaded noise)
            nz = noise_sb[:, k * BPC:(k + 1) * BPC]
            s2 = sp.tile([ACTION_DIM, BPC], f32, tag="s2", name="s2")
            nc.vector.scalar_tensor_tensor(s2, xT, p2, nz, MUL, ADD)

            # ---- the 3 hidden layers ----
            hprev = None
            for L, (wd, bias_off) in enumerate((
                    (None, None), (w2, 0), (w3, HIDDEN))):
                z = pp.tile([128, 2 * BPC], mybir.dt.float32, tag=f"z{L}", name=f"z{L}")
                if L == 0:
                    boff = i * 128
                    nc.tensor.matmul(z, cont[0:4, boff:boff + 128], mask4, start=True, stop=False)
                else:
                    boff = (bias_off // HIDDEN) * 128
                    nc.tensor.matmul(z, b23[0:4, boff:boff + 128], mask4, start=True, stop=False)
                for mc in (0, 1):
                    zslice = z[:, mc * BPC:(mc + 1) * BPC]
                    if L == 0:
                        nc.tensor.matmul(zslice, w1x_hi[:, mc * 128:(mc + 1) * 128], hT, start=False, stop=False)
                        nc.tensor.matmul(zslice, w1x_lo[:, mc * 128:(mc + 1) * 128], hT, start=False, stop=True)
                    else:
                        for kc in (0, 1):
                            rhs = hprev[:, kc * BPC:(kc + 1) * BPC]
                            nc.tensor.matmul(zslice, wd[("hi", kc)][:, mc * 128:(mc + 1) * 128], rhs,
                                             start=False, stop=(kc == 1))
                # sigmoid pass: s = sigmoid(-(A_S*z + D_S))
                s = ap_.tile([128, 2 * BPC], mybir.dt.float32, tag="s", name="s")
                nc.scalar.activation(s, z, SIG, bias=sig_bias, scale=-A_S)
                # custom completion: h = z*QUAD(t)*CUBIC(t), t = s^2
                wA = ap_.tile([128, 2 * BPC], mybir.dt.float32, tag="wA", name="wA")
                nc.vector._custom_dve(MISH_A, out=wA, in0=s, in1=z, s0=QA, s1=QB, imm2=QC)
                h = ap_.tile([128, 2 * BPC], bf, tag=f"h{L}", name=f"h{L}")
                nc.vector._custom_dve(MISH_B, out=h, in0=s, in1=wA, s0=MA, s1=MB, imm2=MC)
                hprev = h

            # ---- L4: eps psum [16, BPC] ----
            z4 = pp.tile([ACTION_DIM, BPC], mybir.dt.float32, tag="z4", name="z4")
            nc.tensor.matmul(z4, w4[("hi", 0)], hprev[:, 0:BPC], start=True, stop=False)
            nc.tensor.matmul(z4, w4[("hi", 1)], hprev[:, BPC:2 * BPC], start=False, stop=True)

            # ---- x update ----
            pre = sp.tile([ACTION_DIM, BPC], f32, tag="pre", name="pre")
            nc.vector._custom_dve(PREOP, out=pre, in0=z4, in1=xT,
                                  s0=xb[:, i:i + 1], s1=c1, imm2=-c2)
            # xT <- clip(pre, -1, 1)*p1 + s2
            nc.vector._custom_dve(CLIPMULADD, out=xT, in0=pre, in1=s2,
                                  s0=-1.0, s1=1.0, imm2=p1)

        out_f = sp.tile([ACTION_DIM, BPC], f32, tag="out_f", name="out_f")
        nc.vector.tensor_scalar(out_f, xT, -1.0, 1.0, MAX, MIN)
        nc.sync.dma_start(d_out, out_f)

    nc.compile()
    _CACHE[('nc', nsteps)] = nc
    return nc


# ---------------------------------------------------------------- host side
def _host_prep(inputs):
    sched = _vp_schedule()
    f64 = np.float64

    W1 = np.asarray(inputs['W1'], np.float32)
    b1 = np.asarray(inputs['b1'], np.float32)
    W2 = np.asarray(inputs['W2'], np.float32)
    b2 = np.asarray(inputs['b2'], np.float32)
    W3 = np.asarray(inputs['W3'], np.float32)
    b3 = np.asarray(inputs['b3'], np.float32)
    W4 = np.asarray(inputs['W4'], np.float32)
    b4 = np.asarray(inputs['b4'], np.float32)

    # time-embedding MLP for all 100 timesteps (host, float64)
    half = TIME_DIM // 2
    freqs = np.exp(np.arange(half, dtype=f64) * (-math.log(10000.0) / (half - 1)))
    ivals = np.arange(T_STEPS, dtype=f64)
    ang = ivals[:, None] * freqs[None, :]
    emb = np.concatenate([np.sin(ang), np.cos(ang)], axis=1)
    t1 = _mish64(emb @ np.asarray(inputs['time_W1'], f64) + np.asarray(inputs['time_b1'], f64))
    temb = t1 @ np.asarray(inputs['time_W2'], f64) + np.asarray(inputs['time_b2'], f64)

    # beta-folded biases
    b2e = b2.astype(f64) + BETA * W2.astype(f64).sum(axis=0)
    b3e = b3.astype(f64) + BETA * W3.astype(f64).sum(axis=0)
    b4e = b4.astype(f64) + BETA * W4.astype(f64).sum(axis=0)

    # contrib[i] = temb[i] @ W1[16:48] + b1   -> flat [1, 100*256]
    contrib = (temb @ W1[16:48].astype(f64) + b1.astype(f64))  # [100, 256]

    def hilo(v):
        v32 = np.asarray(v, np.float32)
        hi = v32.astype(BF16)
        lo = (v32 - hi.astype(np.float32)).astype(BF16)
        return hi, lo

    def pack4(v2d):
        # v2d [G, 256] -> [4, G*128]: rows (hi_a, lo_a, hi_b, lo_b)
        hi, lo = hilo(v2d)
        hi = hi.astype(np.float32); lo = lo.astype(np.float32)
        out = np.stack([hi[:, :128], lo[:, :128], hi[:, 128:], lo[:, 128:]], axis=0)
        return out.reshape(4, -1).astype(BF16)
    cont_hl = pack4(contrib.astype(np.float32))
    b23_hl = pack4(np.stack([b2e, b3e]).astype(np.float32))
    mask4 = np.zeros((4, 2 * BPC), np.float32)
    mask4[0:2, :BPC] = 1.0
    mask4[2:4, BPC:] = 1.0
    mask4 = mask4.astype(BF16)
    w1x = np.concatenate([W1[0:16], W1[48:112]], axis=0)
    w1x_hi, w1x_lo = hilo(w1x)
    w2_hi = np.asarray(W2, np.float32).astype(BF16)
    w3_hi = np.asarray(W3, np.float32).astype(BF16)
    w4_hi = np.asarray(W4, np.float32).astype(BF16)

    # x-update tables
    xb = (-sched['c2'].astype(f64)[None, :] * b4e[:, None]).astype(np.float32)  # [16, 100]

    # per-step noise scaling (fp32, matching the reference ops)
    sig = np.exp(0.5 * sched['logvar']).astype(np.float32)  # [100] by timestep i
    ik = (T_STEPS - 1 - np.arange(T_STEPS))                 # timestep for step k
    scale = sig[ik] * (ik != 0).astype(np.float32)          # [100]
    noise = np.asarray(inputs['noise'], np.float32)
    noise_scaled = noise * scale[:, None, None]

    state = np.asarray(inputs['state'], np.float32)
    x_init = np.asarray(inputs['x_init'], np.float32)

    shared = dict(
        w1x_hi=w1x_hi, w1x_lo=w1x_lo, w2_hi=w2_hi,
        w3_hi=w3_hi, w4_hi=w4_hi,
        cont_hl=cont_hl, b23_hl=b23_hl, mask4=mask4,
        xb_t=xb,
    )
    in_maps = []
    for c in range(NCORES):
        sl = slice(c * BPC, (c + 1) * BPC)
        m = dict(shared)
        m['state_t'] = np.ascontiguousarray(state[sl].T).astype(BF16)
        m['x_init_t'] = np.ascontiguousarray(x_init[sl].T)
        m['noise_t'] = np.ascontiguousarray(noise_scaled[:, sl, :].transpose(0, 2, 1))
        in_maps.append(m)
    return in_maps


def run(inputs, trace=False, nsteps=T_STEPS):
    nc = _build(nsteps)
    in_maps = _host_prep(inputs)
    res = bass_utils.run_bass_kernel_spmd(
        nc, in_maps, core_ids=list(range(NCORES)), trace=trace)
    out = np.empty((BATCH, ACTION_DIM), np.float32)
    for c in range(NCORES):
        out[c * BPC:(c + 1) * BPC] = res.results[c]['out_t'].T
    return out, res


def kernel(**inputs) -> np.ndarray:
    out, _ = run(inputs, trace=False)
    return out


# revision 14
# speedup vs baseline: 1.8509x; 1.0424x over previous
"""Trainium2 Bass kernel for nn_Diffusion_15436112462451.

Strategy: pure data parallelism over the batch (2048 -> 8 cores x 256),
feature-major activations on-chip, fully unrolled 100-step loop.

Per step (per core):
  - 36 bf16 matmuls on PE: split-precision weights (W = Whi + Wlo in bf16,
    two matmuls accumulating in fp32 PSUM) + rank-1 bias matmuls.
  - 3 sigmoid passes on ScalarE (one per hidden layer).
  - 2 fused custom-DVE passes per layer evaluate the exact-mish rational
    completion  mish(z) ~= z * QUAD(t) * CUBIC(t) + beta,  t = sigmoid(-az-d)^2
    (degree-5 minimax fit of tanh(softplus), max err 6.3e-5; beta is folded
    into the next layer's bias on the host).
  - The denoising x-update runs on small [16,256] DVE ops with per-step
    schedule scalars baked in as immediates.

The time-embedding MLP is batch-independent (the timestep is a scalar per
step), so its contribution is precomputed on the host into a [100,256] bias
table and injected via rank-1 bias matmuls.
"""
import sys
import math
import re
import numpy as np

for _p in ('/opt/trn_rl_repo', '/root/.axon_site/_ro/trn_rl_repo'):
    if _p not in sys.path:
        sys.path.insert(0, _p)

import ml_dtypes
from contextlib import ExitStack
import concourse.bass as bass
from concourse import bacc
from concourse import mybir, tile, bass_utils, dve_ops
from concourse.dve_spec import Spec, Src0, Src1, C0, C1, C2, sq, maxx, minn

BF16 = ml_dtypes.bfloat16
NCORES = 8
BATCH = 2048
BPC = BATCH // NCORES          # 256 batch rows per core
T_STEPS = 100
STATE_DIM, ACTION_DIM, HIDDEN, TIME_DIM = 64, 16, 256, 32
KX = ACTION_DIM + STATE_DIM    # 80 rows of W1 used for [x; state]

# --- activation fit constants (deg-5 sigma-poly factorization) ---
A_S = 0.9990298806699722
D_S = -0.0005000143935776705
BETA = 4.708088756431602e-05
QA, QB, QC = -0.21302398380145082, 0.6455208072356895, -0.6201860532189531
MA, MB, MC = -0.9194163848641597, 1.5334239721923986, -1.6124382654378613


# ---------------------------------------------------------------- custom ops
def _register_op(name, spec):
    for op in dve_ops.OPS:
        if op.name == name:
            return op
    op = dve_ops.DveOp(name, spec, False, uops_sha={"v3": "?", "v4": "?"})
    dve_ops.OPS.append(op)
    dve_ops.CUSTOM_DVE_SPECS[name] = spec
    dve_ops._SUB_OPCODE_FOR_NAME[name] = (
        dve_ops._CUSTOM_DVE_ROW_BASE + len(dve_ops.OPS) - 1)
    for ver in ("v3", "v4"):
        try:
            op.compile(ver)
        except ValueError as e:
            op.uops_sha[ver] = re.search(
                r'uops_sha\["' + ver + r'"\]="([0-9a-f]+)"', str(e)).group(1)
        op.compile(ver)
    return op


_t = sq(Src0)
MISH_A = _register_op("MISH_A_DIFF15436", Spec(
    body=Src1 * ((_t * C0 + C1) * _t + C2),
    reference=lambda in0, in1, s0, s1, imm2:
        (in1 * ((s0 * in0.astype(np.float64) ** 2 + s1) * in0.astype(np.float64) ** 2 + imm2)).astype(np.float32),
))
_t2 = sq(Src0)
MISH_B = _register_op("MISH_B_DIFF15436", Spec(
    body=Src1 * ((((_t2 + C0) * _t2 + C1) * _t2) + C2),
    reference=lambda in0, in1, s0, s1, imm2:
        (in1 * ((((in0.astype(np.float64) ** 2 + s0) * in0.astype(np.float64) ** 2 + s1) * in0.astype(np.float64) ** 2) + imm2)).astype(np.float32),
))
PREOP = _register_op("PREOP_DIFF15436", Spec(
    body=Src0 * C2 + Src1 * C1 + C0,
    reference=lambda in0, in1, s0, s1, imm2:
        (in0 * imm2 + in1 * s1 + s0).astype(np.float32),
))
CLIPMULADD = _register_op("CLIPMULADD_DIFF15436", Spec(
    body=minn(maxx(Src0, C0), C1) * C2 + Src1,
    reference=lambda in0, in1, s0, s1, imm2:
        (np.minimum(np.maximum(in0, s0), s1) * imm2 + in1).astype(np.float32),
))


# ---------------------------------------------------------------- schedule
def _vp_schedule():
    t = np.arange(1, T_STEPS + 1, dtype=np.float64)
    b_max, b_min = 10.0, 0.1
    alpha = np.exp(-b_min / T_STEPS - 0.5 * (b_max - b_min) * (2 * t - 1) / T_STEPS ** 2)
    betas = 1.0 - alpha
    ac = np.cumprod(1.0 - betas)
    ac_prev = np.concatenate([[1.0], ac[:-1]])
    return {
        'c1': np.sqrt(1.0 / ac).astype(np.float32),
        'c2': np.sqrt(1.0 / ac - 1.0).astype(np.float32),
        'p1': (betas * np.sqrt(ac_prev) / (1.0 - ac)).astype(np.float32),
        'p2': ((1.0 - ac_prev) * np.sqrt(1.0 - betas) / (1.0 - ac)).astype(np.float32),
        'logvar': np.log(np.clip(betas * (1.0 - ac_prev) / (1.0 - ac), 1e-20, None)).astype(np.float32),
    }


def _mish64(v):
    return v * np.tanh(np.logaddexp(0.0, v))


# ---------------------------------------------------------------- bass build
_CACHE = {}


def _build(nsteps=T_STEPS):
    if ('nc', nsteps) in _CACHE:
        return _CACHE[('nc', nsteps)]
    sched = _vp_schedule()
    c1s, c2s, p1s, p2s = sched['c1'], sched['c2'], sched['p1'], sched['p2']

    nc = bacc.Bacc("TRN2", target_bir_lowering=False, debug=False, num_devices=NCORES)
    f32 = mybir.dt.float32
    bf = mybir.dt.bfloat16

    def din(name, shape, dt=f32):
        return nc.dram_tensor(name, shape, dt, kind="ExternalInput").ap()

    d_state = din("state_t", [STATE_DIM, BPC], bf)
    d_xinit = din("x_init_t", [ACTION_DIM, BPC])
    d_noise = din("noise_t", [T_STEPS, ACTION_DIM, BPC])
    d_w1x_hi = din("w1x_hi", [KX, HIDDEN], bf)
    d_w1x_lo = din("w1x_lo", [KX, HIDDEN], bf)
    d_w2_hi = din("w2_hi", [HIDDEN, HIDDEN], bf)
    d_w3_hi = din("w3_hi", [HIDDEN, HIDDEN], bf)
    d_w4_hi = din("w4_hi", [HIDDEN, ACTION_DIM], bf)
    d_cont = din("cont_hl", [4, T_STEPS * 128], bf)
    d_b23 = din("b23_hl", [4, 2 * 128], bf)
    d_mask = din("mask4", [4, 2 * BPC], bf)
    d_xb = din("xb_t", [ACTION_DIM, T_STEPS])
    d_out = nc.dram_tensor("out_t", [ACTION_DIM, BPC], f32, kind="ExternalOutput").ap()

    with tile.TileContext(nc) as tc, ExitStack() as ctx:
        wp = ctx.enter_context(tc.tile_pool(name="weights", bufs=1))
        ap_ = ctx.enter_context(tc.tile_pool(name="acts", bufs=2))
        sp = ctx.enter_context(tc.tile_pool(name="small", bufs=2))
        np_ = ctx.enter_context(tc.tile_pool(name="noise", bufs=4))
        pp = ctx.enter_context(tc.tile_pool(name="psum", bufs=2, space="PSUM"))

        def wtile(shape, dt, nm, src):
            t = wp.tile(shape, dt, tag=nm, name=nm)
            nc.gpsimd.dma_start(t, src)
            return t

        w1x_hi = wtile([KX, HIDDEN], bf, "w1x_hi", d_w1x_hi)
        w1x_lo = wtile([KX, HIDDEN], bf, "w1x_lo", d_w1x_lo)
        w2 = {}
        w3 = {}
        w4 = {}
        for nm, dhi, dst in (("w2", d_w2_hi, w2), ("w3", d_w3_hi, w3)):
            for kc in (0, 1):
                dst[("hi", kc)] = wtile([128, HIDDEN], bf, f"{nm}_hi_{kc}",
                                        dhi[kc * 128:(kc + 1) * 128, :])
        for kc in (0, 1):
            w4[("hi", kc)] = wtile([128, ACTION_DIM], bf, f"w4_hi_{kc}",
                                   d_w4_hi[kc * 128:(kc + 1) * 128, :])
        cont = wtile([4, T_STEPS * 128], bf, "cont", d_cont)
        b23 = wtile([4, 2 * 128], bf, "b23", d_b23)
        mask4 = wtile([4, 2 * BPC], bf, "mask4", d_mask)
        noise_sb = wp.tile([ACTION_DIM, T_STEPS * BPC], f32, tag="noise_sb", name="noise_sb")
        nc.gpsimd.dma_start(
            noise_sb.rearrange("p (k c) -> p k c", k=T_STEPS),
            d_noise.rearrange("k p c -> p k c"))
        xb = wtile([ACTION_DIM, T_STEPS], f32, "xb", d_xb)

        sig_bias = wp.tile([128, 1], f32, tag="sig_bias", name="sig_bias")
        nc.vector.memset(sig_bias, -D_S)

        hT = wp.tile([KX, BPC], bf, tag="hT", name="hT")
        nc.gpsimd.dma_start(hT[ACTION_DIM:KX, :], d_state)
        xT = wp.tile([ACTION_DIM, BPC], f32, tag="xT", name="xT")
        nc.gpsimd.dma_start(xT, d_xinit)
        nc.vector.tensor_copy(hT[0:ACTION_DIM, :], xT)

        SIG = mybir.ActivationFunctionType.Sigmoid
        MUL = mybir.AluOpType.mult
        ADD = mybir.AluOpType.add
        MAX = mybir.AluOpType.max
        MIN = mybir.AluOpType.min

        for k in range(nsteps):
            i = T_STEPS - 1 - k
            c1 = float(c1s[i]); c2 = float(c2s[i])
            p1 = float(p1s[i]); p2 = float(p2s[i])

            # early elementwise pieces (only depend on x_k and preloaded noise)
            nz = noise_sb[:, k * BPC:(k + 1) * BPC]
            s2 = sp.tile([ACTION_DIM, BPC], f32, tag="s2", name="s2")
            nc.vector.scalar_tensor_tensor(s2, xT, p2, nz, MUL, ADD)

            # ---- the 3 hidden layers ----
            hprev = None
            for L, (wd, bias_off) in enumerate((
                    (None, None), (w2, 0), (w3, HIDDEN))):
                z = pp.tile([128, 2 * BPC], mybir.dt.float32, tag=f"z{L}", name=f"z{L}")
                if L == 0:
                    boff = i * 128
                    nc.tensor.matmul(z, cont[0:4, boff:boff + 128], mask4, start=True, stop=False)
                else:
                    boff = (bias_off // HIDDEN) * 128
                    nc.tensor.matmul(z, b23[0:4, boff:boff + 128], mask4, start=True, stop=False)
                for mc in (0, 1):
                    zslice = z[:, mc * BPC:(mc + 1) * BPC]
                    if L == 0:
                        nc.tensor.matmul(zslice, w1x_hi[:, mc * 128:(mc + 1) * 128], hT, start=False, stop=False)
                        nc.tensor.matmul(zslice, w1x_lo[:, mc * 128:(mc + 1) * 128], hT, start=False, stop=True)
                    else:
                        for kc in (0, 1):
                            rhs = hprev[:, kc * BPC:(kc + 1) * BPC]
                            nc.tensor.matmul(zslice, wd[("hi", kc)][:, mc * 128:(mc + 1) * 128], rhs,
                                             start=False, stop=(kc == 1))
                # sigmoid pass: s = sigmoid(-(A_S*z + D_S))
                s = ap_.tile([128, 2 * BPC], mybir.dt.float32, tag="s", name="s")
                nc.scalar.activation(s, z, SIG, bias=sig_bias, scale=-A_S)
                # custom completion: h = z*QUAD(t)*CUBIC(t), t = s^2
                wA = ap_.tile([128, 2 * BPC], mybir.dt.float32, tag="wA", name="wA")
                nc.vector._custom_dve(MISH_A, out=wA, in0=s, in1=z, s0=QA, s1=QB, imm2=QC)
                h = ap_.tile([128, 2 * BPC], bf, tag=f"h{L}", name=f"h{L}")
                nc.vector._custom_dve(MISH_B, out=h, in0=s, in1=wA, s0=MA, s1=MB, imm2=MC)
                hprev = h

            # ---- L4: eps psum [16, BPC] ----
            z4 = pp.tile([ACTION_DIM, BPC], mybir.dt.float32, tag="z4", name="z4")
            nc.tensor.matmul(z4, w4[("hi", 0)], hprev[:, 0:BPC], start=True, stop=False)
            nc.tensor.matmul(z4, w4[("hi", 1)], hprev[:, BPC:2 * BPC], start=False, stop=True)

            # ---- x update ----
            pre = sp.tile([ACTION_DIM, BPC], f32, tag="pre", name="pre")
            nc.vector._custom_dve(PREOP, out=pre, in0=z4, in1=xT,
                                  s0=xb[:, i:i + 1], s1=c1, imm2=-c2)
            # x_{k+1} = clip(pre, -1, 1)*p1 + s2: write the bf16 matmul view
            # first (feeds the next step's L1), then the fp32 master.
            nc.vector._custom_dve(CLIPMULADD, out=hT[0:ACTION_DIM, :], in0=pre, in1=s2,
                                  s0=-1.0, s1=1.0, imm2=p1)
            nc.vector._custom_dve(CLIPMULADD, out=xT, in0=pre, in1=s2,
                                  s0=-1.0, s1=1.0, imm2=p1)

        out_f = sp.tile([ACTION_DIM, BPC], f32, tag="out_f", name="out_f")
        nc.vector.tensor_scalar(out_f, xT, -1.0, 1.0, MAX, MIN)
        nc.sync.dma_start(d_out, out_f)

    nc.compile()
    _CACHE[('nc', nsteps)] = nc
    return nc


# ---------------------------------------------------------------- host side
def _host_prep(inputs):
    sched = _vp_schedule()
    f64 = np.float64

    W1 = np.asarray(inputs['W1'], np.float32)
    b1 = np.asarray(inputs['b1'], np.float32)
    W2 = np.asarray(inputs['W2'], np.float32)
    b2 = np.asarray(inputs['b2'], np.float32)
    W3 = np.asarray(inputs['W3'], np.float32)
    b3 = np.asarray(inputs['b3'], np.float32)
    W4 = np.asarray(inputs['W4'], np.float32)
    b4 = np.asarray(inputs['b4'], np.float32)

    # time-embedding MLP for all 100 timesteps (host, float64)
    half = TIME_DIM // 2
    freqs = np.exp(np.arange(half, dtype=f64) * (-math.log(10000.0) / (half - 1)))
    ivals = np.arange(T_STEPS, dtype=f64)
    ang = ivals[:, None] * freqs[None, :]
    emb = np.concatenate([np.sin(ang), np.cos(ang)], axis=1)
    t1 = _mish64(emb @ np.asarray(inputs['time_W1'], f64) + np.asarray(inputs['time_b1'], f64))
    temb = t1 @ np.asarray(inputs['time_W2'], f64) + np.asarray(inputs['time_b2'], f64)

    # beta-folded biases
    b2e = b2.astype(f64) + BETA * W2.astype(f64).sum(axis=0)
    b3e = b3.astype(f64) + BETA * W3.astype(f64).sum(axis=0)
    b4e = b4.astype(f64) + BETA * W4.astype(f64).sum(axis=0)

    # contrib[i] = temb[i] @ W1[16:48] + b1   -> flat [1, 100*256]
    contrib = (temb @ W1[16:48].astype(f64) + b1.astype(f64))  # [100, 256]

    def hilo(v):
        v32 = np.asarray(v, np.float32)
        hi = v32.astype(BF16)
        lo = (v32 - hi.astype(np.float32)).astype(BF16)
        return hi, lo

    def pack4(v2d):
        # v2d [G, 256] -> [4, G*128]: rows (hi_a, lo_a, hi_b, lo_b)
        hi, lo = hilo(v2d)
        hi = hi.astype(np.float32); lo = lo.astype(np.float32)
        out = np.stack([hi[:, :128], lo[:, :128], hi[:, 128:], lo[:, 128:]], axis=0)
        return out.reshape(4, -1).astype(BF16)
    cont_hl = pack4(contrib.astype(np.float32))
    b23_hl = pack4(np.stack([b2e, b3e]).astype(np.float32))
    mask4 = np.zeros((4, 2 * BPC), np.float32)
    mask4[0:2, :BPC] = 1.0
    mask4[2:4, BPC:] = 1.0
    mask4 = mask4.astype(BF16)
    w1x = np.concatenate([W1[0:16], W1[48:112]], axis=0)
    w1x_hi, w1x_lo = hilo(w1x)
    w2_hi = np.asarray(W2, np.float32).astype(BF16)
    w3_hi = np.asarray(W3, np.float32).astype(BF16)
    w4_hi = np.asarray(W4, np.float32).astype(BF16)

    # x-update tables
    xb = (-sched['c2'].astype(f64)[None, :] * b4e[:, None]).astype(np.float32)  # [16, 100]

    # per-step noise scaling (fp32, matching the reference ops)
    sig = np.exp(0.5 * sched['logvar']).astype(np.float32)  # [100] by timestep i
    ik = (T_STEPS - 1 - np.arange(T_STEPS))                 # timestep for step k
    scale = sig[ik] * (ik != 0).astype(np.float32)          # [100]
    noise = np.asarray(inputs['noise'], np.float32)
    noise_scaled = noise * scale[:, None, None]

    state = np.asarray(inputs['state'], np.float32)
    x_init = np.asarray(inputs['x_init'], np.float32)

    shared = dict(
        w1x_hi=w1x_hi, w1x_lo=w1x_lo, w2_hi=w2_hi,
        w3_hi=w3_hi, w4_hi=w4_hi,
        cont_hl=cont_hl, b23_hl=b23_hl, mask4=mask4,
        xb_t=xb,
    )
    in_maps = []
    for c in range(NCORES):
        sl = slice(c * BPC, (c + 1) * BPC)
        m = dict(shared)
        m['state_t'] = np.ascontiguousarray(state[sl].T).astype(BF16)
        m['x_init_t'] = np.ascontiguousarray(x_init[sl].T)
        m['noise_t'] = np.ascontiguousarray(noise_scaled[:, sl, :].transpose(0, 2, 1))
        in_maps.append(m)
    return in_maps


def run(inputs, trace=False, nsteps=T_STEPS):
    nc = _build(nsteps)
    in_maps = _host_prep(inputs)
    res = bass_utils.run_bass_kernel_spmd(
        nc, in_maps, core_ids=list(range(NCORES)), trace=trace)
    out = np.empty((BATCH, ACTION_DIM), np.float32)
    for c in range(NCORES):
        out[c * BPC:(c + 1) * BPC] = res.results[c]['out_t'].T
    return out, res


def kernel(**inputs) -> np.ndarray:
    out, _ = run(inputs, trace=False)
    return out
